# revision 18
# baseline (speedup 1.0000x reference)
"""Trainium2 8-core kernel for nn_EnhancedTransformerBlock (v2).

SPMD: identical program on all 8 cores.
  - Full x replicated to every core (bf16) -> no AllGather. Each core
    computes LN stats for all 2048 tokens, normalizes x, then QKV for its
    2 heads (head-sharded attention over the full sequence).
  - ln_attn affine and softmax temperature folded into QKV weights host-side.
  - Entropy gate folded into the V GEMM as an extra output column.
  - Attention: unshifted exp, denominator via ones-column on V, causal
    triangle masks, single reciprocal for all 8 (head, q-group) denominators.
  - AllToAll of per-head attention outputs back to sequence sharding
    (core c owns tokens [256c, 256c+256) for the FFN part).
  - ff1/ff2 GEMMs in fp8(e4m3) DoubleRow mode (K=256 per instruction,
    2x rate); activations scaled x16/x64 to sit in fp8's normal range.
  - Spline activation approximated by a 4-term kink-basis LSQ fit of the
    fixed 1-D function g(u) (computed host-side from runtime knots/spl_w);
    evaluated in ~9 elementwise ops per 2048-col group, alternating
    Vector/GpSimd engines; ff2 partial GEMMs interleaved with spline groups.
"""

import hashlib
import numpy as np

from concourse import bacc, tile, mybir
from concourse import bass_utils

dt = mybir.dt
BF = dt.bfloat16
F32 = dt.float32
FP8 = dt.float8e4
NPBF = dt.np(BF)
NPF8 = dt.np(FP8)
Alu = mybir.AluOpType
Act = mybir.ActivationFunctionType
PM = mybir.MatmulPerfMode

NCORES = 8
S = 2048
D = 1024
H = 16
HD = 64
FF = 4096
D16 = 256
TOK = S // NCORES            # 256 tokens per core
HPC = H // NCORES            # 2 heads per core
EPS = 1e-6
UDOM = 0.12                  # spline fit domain |u| <= UDOM (|u| < 0.09 true)

SX1 = 16.0                   # x1 fp8 scale
SW1 = 64.0                   # ff1_w fp8 scale
SA = 64.0                    # act fp8 scale
SW2 = 64.0                   # ff2_w fp8 scale
SQ = 4096.0                  # spline-delta fp8 scale

_prog_cache = {}


# ----------------------------------------------------------------------------
# Host-side: spline fit
# ----------------------------------------------------------------------------

def _g_exact(u, knots, spl_w):
    d = np.abs(u[:, None] - knots[None, :])
    d = d / (d.max(-1, keepdims=True) + EPS)
    a = -5.0 * d
    a = a - a.max(-1, keepdims=True)
    e = np.exp(a)
    p = e / e.sum(-1, keepdims=True)
    return (p * spl_w).sum(-1)


def _fit_spline(knots, spl_w):
    """LSQ fit of g(u) on [-UDOM, UDOM]; basis [1, u, u^2, |u|]."""
    k = np.asarray(knots, np.float64)
    w = np.asarray(spl_w, np.float64)
    u = np.linspace(-UDOM, UDOM, 20001)
    B = np.stack([np.ones_like(u), u, u * u, np.abs(u)], -1)
    y = _g_exact(u, k, w)
    c, *_ = np.linalg.lstsq(B, y, rcond=None)
    err = float(np.abs(B @ c - y).max())
    return [float(v) for v in c], err


# ----------------------------------------------------------------------------
# Host-side: weight packing
# ----------------------------------------------------------------------------

def _pack_lhsT(w_t, n_of, n_kc):
    """w_t: [K_total, M_total] ([in, out]) -> [128, n_of*n_kc*128], tile
    (of, kc) at cols [(of*n_kc+kc)*128 ...] = w_t[128kc:.., 128of:..]."""
    K_total, M_total = w_t.shape
    assert K_total == n_kc * 128 and M_total == n_of * 128
    out = np.empty((128, n_of * n_kc * 128), np.float32)
    for of in range(n_of):
        for kc in range(n_kc):
            out[:, (of * n_kc + kc) * 128:(of * n_kc + kc + 1) * 128] = \
                w_t[kc * 128:(kc + 1) * 128, of * 128:(of + 1) * 128]
    return np.ascontiguousarray(out)


def _col_pack(vec, n_chunks):
    return np.ascontiguousarray(
        np.asarray(vec, np.float32).reshape(n_chunks, 128).T)


def _make_tri_masks():
    out = np.zeros((128, 4 * 512), np.float32)
    for j in range(4):
        kk = np.arange(128)[:, None] + 128 * j
        q = np.arange(512)[None, :]
        out[:, 512 * j:512 * (j + 1)] = (kk <= q).astype(np.float32)
    return out


# const blob layout (f32 [128, CW]); col offsets
_CO = {}
_cw = 0
def _co(name, w):
    global _cw
    _CO[name] = _cw
    _cw += w
_co("b_qk", 2)
_co("b_out", 8)
_co("b_ff1", 32)
_co("b_ff2", 8)
_co("b_ep1", 2)
_co("n1w", 8)
_co("n1b", 8)
_co("n1w16", 8)
_co("n1b16", 8)
_co("n2w", 8)
_co("n2b", 8)
_co("eplw", 2)
_co("eplb", 2)
_co("rsw2", 8)
_co("bv_row", 137)           # row 0 only
CW = _cw


def _prepare_inputs(inputs):
    f = lambda k: np.asarray(inputs[k], np.float32)
    x = f("x").reshape(S, D)
    qkv_w, qkv_b = f("qkv_w"), f("qkv_b")
    out_w, out_b = f("out_w") * 0.1, f("out_b") * 0.1
    ff1_w, ff1_b = f("ff1_w"), f("ff1_b")
    ff2_w, ff2_b = f("ff2_w"), f("ff2_b")
    ep1_w, ep1_b = f("ep1_w"), f("ep1_b")
    ep2_w, ep2_b = f("ep2_w"), f("ep2_b")
    ent_w, ent_b = f("ent_w"), f("ent_b")
    lnw, lnb = f("ln_attn_w"), f("ln_attn_b")
    n1w, n1b = f("norm1_w"), f("norm1_b")

    temp = (1.0 / np.sqrt(np.float32(HD))) / 0.1   # 1.25
    # fold ln_attn affine into qkv/ent weights: W'(xn) + b' == W(xl) + b
    wq = qkv_w[0:D] * temp * lnw[None, :]
    wk = qkv_w[D:2 * D] * lnw[None, :]
    wv = qkv_w[2 * D:3 * D] * lnw[None, :]
    bq = qkv_b[0:D] * temp + wq @ lnb
    bk = qkv_b[D:2 * D] + wk @ lnb
    bv = qkv_b[2 * D:3 * D] + wv @ lnb
    went = ent_w.reshape(D) * lnw
    bent = float(ent_b.reshape(-1)[0] + went @ lnb)

    coeffs, fit_err = _fit_spline(f("knots"), f("spl_w"))

    xT = np.ascontiguousarray(x.T)                           # [D, S]
    xfull = np.ascontiguousarray(
        xT.reshape(8, 128, S).transpose(1, 0, 2).reshape(128, 8 * S)
    ).astype(NPBF)

    shared = {
        "xfull": xfull,
        "tri": _make_tri_masks().astype(NPBF),
        "wout": _pack_lhsT(out_w.T, 8, 8).astype(NPBF),
        "wff1": np.ascontiguousarray(
            _pack_lhsT(ff1_w.T * SW1, 32, 8)).astype(NPF8),
        "wff2": np.ascontiguousarray(
            _pack_lhsT(ff2_w.T * SW2, 8, 32)).astype(NPF8),
        "wep1": _pack_lhsT(ep1_w.T, 2, 32).astype(NPBF),
        "wep2": np.ascontiguousarray(
            ep2_w.reshape(2, 128).T).astype(NPBF),          # [128, 2]
    }

    scalars = {
        "ent_b": bent,
        "ep2_b": float(ep2_b.reshape(-1)[0]),
        "coeffs": coeffs,
        "fit_err": fit_err,
    }

    in_maps = []
    for c in range(NCORES):
        m = dict(shared)
        xc = x[c * TOK:(c + 1) * TOK]                        # [256, D]
        xTc = np.ascontiguousarray(xc.T)                     # [D, 256]
        m["xT"] = np.ascontiguousarray(
            xTc.reshape(8, 128, TOK).transpose(1, 0, 2).reshape(128, 8 * TOK))
        h0 = c * HPC
        wq_c = wq[h0 * HD:(h0 + HPC) * HD]                   # [128, D]
        wk_c = wk[h0 * HD:(h0 + HPC) * HD]
        wqk_t = np.concatenate([wq_c, wk_c], 0).T            # [D, 256]
        m["wqk"] = _pack_lhsT(wqk_t, 2, 8).astype(NPBF)
        # V weights [D, 137]: v_h0(64) pad(4) v_h1(64) pad(4) went(1)
        wv_c = wv[h0 * HD:(h0 + HPC) * HD].T                 # [D, 128]
        wva = np.zeros((D, 137), np.float32)
        bva = np.zeros((1, 137), np.float32)
        for lh in range(HPC):
            wva[:, 68 * lh:68 * lh + 64] = wv_c[:, 64 * lh:64 * lh + 64]
            bva[0, 68 * lh:68 * lh + 64] = \
                bv[(h0 + lh) * HD:(h0 + lh + 1) * HD]
        wva[:, 136] = went
        m["wv"] = np.ascontiguousarray(
            wva.reshape(8, 128, 137).transpose(1, 0, 2).reshape(128, 8 * 137)
        ).astype(NPBF)
        # const blob
        blob = np.zeros((128, CW), np.float32)
        def put(name, arr):
            a = np.asarray(arr, np.float32)
            blob[:, _CO[name]:_CO[name] + a.shape[1]] = a
        put("b_qk", np.stack([bq[h0 * HD:(h0 + HPC) * HD],
                              bk[h0 * HD:(h0 + HPC) * HD]], -1))
        put("b_out", _col_pack(out_b, 8))
        put("b_ff1", _col_pack(ff1_b, 32))
        put("b_ff2", _col_pack(ff2_b, 8))
        put("b_ep1", _col_pack(ep1_b, 2))
        put("n1w", _col_pack(n1w, 8))
        put("n1b", _col_pack(n1b, 8))
        put("n1w16", _col_pack(n1w * SX1, 8))
        put("n1b16", _col_pack(n1b * SX1, 8))
        put("n2w", _col_pack(f("norm2_w"), 8))
        put("n2b", _col_pack(f("norm2_b"), 8))
        put("eplw", _col_pack(f("ep_ln_w"), 2))
        put("eplb", _col_pack(f("ep_ln_b"), 2))
        put("rsw2", _col_pack(ff2_w.sum(1) / SA, 8))
        blob[0, _CO["bv_row"]:_CO["bv_row"] + 137] = bva[0]
        m["blob"] = np.ascontiguousarray(blob)
        in_maps.append(m)

    return in_maps, scalars


# ----------------------------------------------------------------------------
# Device program
# ----------------------------------------------------------------------------

def _build_program(sc):
    nc = bacc.Bacc("TRN2", target_bir_lowering=False, debug=False,
                   num_devices=NCORES)

    def din(name, shape, dtype):
        return nc.dram_tensor(name, list(shape), dtype, kind="ExternalInput")

    tin = {
        "xfull": din("xfull", (128, 8 * S), BF),
        "xT": din("xT", (128, 8 * TOK), F32),
        "wqk": din("wqk", (128, 2048), BF),
        "wv": din("wv", (128, 8 * 137), BF),
        "wout": din("wout", (128, 8192), BF),
        "wff1": din("wff1", (128, 32768), FP8),
        "wff2": din("wff2", (128, 32768), FP8),
        "wep1": din("wep1", (128, 8192), BF),
        "wep2": din("wep2", (128, 2), BF),
        "tri": din("tri", (128, 2048), BF),
        "blob": din("blob", (128, CW), F32),
    }
    t_out = nc.dram_tensor("out", [128, 8 * TOK], F32, kind="ExternalOutput")
    a2a_in = nc.dram_tensor("a2a_in", [1024, TOK], BF, kind="Internal")
    a2a_out = nc.dram_tensor("a2a_out", [1024, TOK], BF, kind="Internal")

    with tile.TileContext(nc) as tc:
        _emit(nc, tc, tin, t_out, a2a_in, a2a_out, sc)
    nc.compile()
    return nc


def _ln_rows(nc, mu, st, tmp, sx, sx2, n, epsap):
    """mu = sx/n; st = 1/sqrt(var+eps) with var = sx2/n - mu^2."""
    v, s = nc.vector, nc.scalar
    v.tensor_scalar(mu, sx, 1.0 / n, None, Alu.mult)
    v.tensor_tensor(st, mu, mu, Alu.mult)
    v.tensor_scalar(tmp, sx2, 1.0 / n, None, Alu.mult)
    v.tensor_tensor(st, tmp, st, Alu.subtract)
    s.activation(st, st, Act.Ln, bias=epsap)
    s.activation(st, st, Act.Exp, scale=-0.5)


def _emit(nc, tc, tin, t_out, a2a_in, a2a_out, sc):
    v = nc.vector
    s = nc.scalar
    g = nc.gpsimd
    te = nc.tensor
    c0, c1, c2, c3 = sc["coeffs"]
    RG = [list(range(NCORES))]

    with tc.tile_pool(name="persist", bufs=1) as P, \
         tc.tile_pool(name="consts", bufs=1) as C, \
         tc.tile_pool(name="rows", bufs=1) as R:

        # ---- constants + input DMAs, spread across queues ----
        blob = C.tile([128, CW], F32, tag="blob")
        nc.sync.dma_start(out=blob[:], in_=tin["blob"].ap())
        blc = lambda nm, k: blob[:, _CO[nm] + k:_CO[nm] + k + 1]

        tri = C.tile([128, 2048], BF, tag="tri")
        g.dma_start(out=tri[:], in_=tin["tri"].ap())
        xt = P.tile([128, 8 * TOK], F32, tag="xt")
        g.dma_start(out=xt[:], in_=tin["xT"].ap())

        XN_cm = tc.tile_pool(name="xn_pool", bufs=1)
        XN = XN_cm.__enter__()
        xn = XN.tile([128, 8 * S], BF, tag="xn")
        XF_cm = tc.tile_pool(name="xf_pool", bufs=1)
        XF = XF_cm.__enter__()
        xf = XF.tile([128, 8 * S], BF, tag="xf")
        qdma = [nc.sync.dma_start, nc.scalar.dma_start,
                nc.gpsimd.dma_start, nc.sync.dma_start]
        for q in range(4):
            qdma[q](out=xf[:, 4096 * q:4096 * (q + 1)],
                    in_=tin["xfull"].ap()[:, 4096 * q:4096 * (q + 1)])

        onesb = C.tile([128, 1], BF, tag="onesb")
        ones32 = C.tile([128, 1], F32, tag="ones32")
        onesr = C.tile([1, 64], BF, tag="onesr")
        v.memset(onesb[:], 1.0)
        v.memset(ones32[:], 1.0)
        v.memset(onesr[:], 1.0)
        cst = C.tile([128, 4], F32, tag="cst")
        v.memset(cst[:, 0:1], EPS)
        v.memset(cst[:, 1:2], sc["ent_b"])
        v.memset(cst[:, 2:3], sc["ep2_b"])
        bvb = C.tile([128, 137], F32, tag="bvb")
        g.partition_broadcast(bvb[:], blob[0:1, _CO["bv_row"]:_CO["bv_row"] + 137])

        # persistent activations
        qkT = P.tile([128, 4096], BF, tag="qkT")
        vaug = P.tile([128, 16 * 137], BF, tag="vaug")
        aosc = P.tile([128, 2048], BF, tag="aosc")
        aofull = P.tile([128, 8 * TOK], BF, tag="aofull")
        x1f = P.tile([128, 8 * TOK], F32, tag="x1f")
        x1b8 = P.tile([128, 8, TOK], FP8, tag="x1b8")
        rows = R.tile([1, 16 * TOK], F32, tag="rows")
        rs = lambda k: rows[0:1, k * TOK:(k + 1) * TOK]
        denpA = R.tile([128, 512], F32, tag="denpA")
        denpB = R.tile([128, 512], F32, tag="denpB")

        # ============ Phase 1: full-seq LN stats + normalize ============
        with tc.tile_pool(name="ps_r1", bufs=4, space="PSUM") as PSR, \
             tc.tile_pool(name="tmp1", bufs=1) as TMP:
            mu_r = TMP.tile([1, S], F32, tag="mu_r")
            st_r = TMP.tile([1, S], F32, tag="st_r")
            tm_r = TMP.tile([1, S], F32, tag="tm_r")
            for w in range(4):
                sx = PSR.tile([1, 512], F32, tag="sx1p")
                sx2 = PSR.tile([1, 512], F32, tag="sx2p")
                sx, sx2 = sx[:], sx2[:]
                for kc in range(8):
                    te.matmul(sx, onesb[:],
                              xf[:, 2048 * kc + 512 * w:2048 * kc + 512 * (w + 1)],
                              start=(kc == 0), stop=(kc == 7))
                for kc in range(8):
                    xsq = TMP.tile([128, 512], BF, tag="xsq", bufs=4)
                    eng = v if kc % 2 == 0 else g
                    eng.tensor_tensor(
                        xsq[:], xf[:, 2048 * kc + 512 * w:2048 * kc + 512 * (w + 1)],
                        xf[:, 2048 * kc + 512 * w:2048 * kc + 512 * (w + 1)],
                        Alu.mult)
                    te.matmul(sx2, onesb[:], xsq[:],
                              start=(kc == 0), stop=(kc == 7))
                v.tensor_scalar(mu_r[0:1, 512 * w:512 * (w + 1)], sx,
                                1.0 / D, None, Alu.mult)
                v.tensor_scalar(st_r[0:1, 512 * w:512 * (w + 1)], sx2,
                                1.0 / D, None, Alu.mult)
            v.tensor_tensor(tm_r[:], mu_r[:], mu_r[:], Alu.mult)
            v.tensor_tensor(st_r[:], st_r[:], tm_r[:], Alu.subtract)
            s.activation(st_r[:], st_r[:], Act.Ln, bias=cst[0:1, 0:1])
            s.activation(st_r[:], st_r[:], Act.Exp, scale=-0.5)
            v.tensor_tensor(tm_r[:], mu_r[:], st_r[:], Alu.mult)
            # bf16 reps
            str_b = TMP.tile([1, S], BF, tag="str_b")
            mst_b = TMP.tile([1, S], BF, tag="mst_b")
            v.tensor_copy(str_b[:], st_r[:])
            v.tensor_copy(mst_b[:], tm_r[:])
            strep = TMP.tile([128, S], BF, tag="strep")
            mstrep = TMP.tile([128, S], BF, tag="mstrep")
            g.partition_broadcast(strep[:], str_b[:])
            g.partition_broadcast(mstrep[:], mst_b[:])
            for kc in range(8):
                tm = TMP.tile([128, S], BF, tag="nrm", bufs=2)
                eng = v if kc % 2 == 0 else g
                eng.tensor_tensor(tm[:], xf[:, 2048 * kc:2048 * (kc + 1)],
                                  strep[:], Alu.mult)
                eng.tensor_tensor(xn[:, 2048 * kc:2048 * (kc + 1)],
                                  tm[:], mstrep[:], Alu.subtract)
        XF_cm.__exit__(None, None, None)

        # ============ Phase 2: QKV + ent ============
        with tc.tile_pool(name="wq_pool", bufs=1) as WQ, \
             tc.tile_pool(name="ps_qk", bufs=2, space="PSUM") as PSQ, \
             tc.tile_pool(name="ps_ev", bufs=3, space="PSUM") as PSV, \
             tc.tile_pool(name="estmp", bufs=4) as EST:
            wv_s = WQ.tile([128, 8 * 137], BF, tag="wv_s")
            nc.scalar.dma_start(out=wv_s[:], in_=tin["wv"].ap())
            wqk_s = WQ.tile([128, 2048], BF, tag="wqk_s")
            nc.sync.dma_start(out=wqk_s[:], in_=tin["wqk"].ap())

            for tch in range(16):
                psv = PSV.tile([128, 137], F32, tag="psv", bufs=2)
                for kc in range(8):
                    te.matmul(
                        psv[:],
                        xn[:, 2048 * kc + 128 * tch:2048 * kc + 128 * (tch + 1)],
                        wv_s[:, 137 * kc:137 * (kc + 1)],
                        start=(kc == 0), stop=(kc == 7))
                esc = EST.tile([128, 1], F32, tag="esc")
                s.activation(esc[:], psv[:, 136:137], Act.Sigmoid,
                             bias=cst[:, 1:2])
                v.tensor_scalar(esc[:], esc[:], 0.1, None, Alu.max)
                vt = vaug[:, 137 * tch:137 * tch + 136]
                v.tensor_tensor(vt, psv[:, 0:136], bvb[:, 0:136], Alu.add)
                v.tensor_scalar(vt, vt, esc[:], None, Alu.mult)
                for lh in range(HPC):
                    v.memset(vaug[:, 137 * tch + 68 * lh + 64:
                                  137 * tch + 68 * lh + 65], 1.0)

            for of in range(2):
                for w in range(4):
                    ps = PSQ.tile([128, 512], F32, tag="psqk")
                    for kc in range(8):
                        te.matmul(
                            ps[:],
                            wqk_s[:, (of * 8 + kc) * 128:(of * 8 + kc + 1) * 128],
                            xn[:, 2048 * kc + 512 * w:2048 * kc + 512 * (w + 1)],
                            start=(kc == 0), stop=(kc == 7))
                    v.tensor_scalar(
                        qkT[:, 2048 * of + 512 * w:2048 * of + 512 * (w + 1)],
                        ps[:], blc("b_qk", of), None, Alu.add)
        XN_cm.__exit__(None, None, None)

        # ---- prefetch big weights (land during attention) ----
        TMP3_cm = tc.tile_pool(name="tmp3", bufs=1)
        TMP3 = TMP3_cm.__enter__()
        hb = TMP3.tile([128, 8192], BF, tag="hb")
        murep = TMP3.tile([128, 2048], BF, tag="murep")
        Srep = TMP3.tile([128, 2048], BF, tag="Srep")
        emqrep = TMP3.tile([128, 2048], BF, tag="emqrep")
        thrrep = TMP3.tile([128, 2048], BF, tag="thrrep")
        W6_cm = tc.tile_pool(name="w6_pool", bufs=1)
        W6 = W6_cm.__enter__()
        WO_cm = tc.tile_pool(name="wo_pool", bufs=1)
        WO = WO_cm.__enter__()
        wout_s = WO.tile([128, 8192], BF, tag="wout_s")
        wep1_s = W6.tile([128, 8192], BF, tag="wep1_s")
        wff1_s = W6.tile([128, 256, 128], FP8, tag="wff1_s")
        pq = [nc.sync.dma_start, nc.scalar.dma_start, nc.gpsimd.dma_start,
              nc.sync.dma_start]
        for q in range(4):
            pq[q](out=wout_s[:, 2048 * q:2048 * (q + 1)],
                  in_=tin["wout"].ap()[:, 2048 * q:2048 * (q + 1)])
        for q in range(4):
            pq[q](out=wff1_s[:, 64 * q:64 * (q + 1), :],
                  in_=tin["wff1"].ap()[:, 8192 * q:8192 * (q + 1)])
        for q in range(4):
            pq[q](out=wep1_s[:, 2048 * q:2048 * (q + 1)],
                  in_=tin["wep1"].ap()[:, 2048 * q:2048 * (q + 1)])

        # ============ Phase 3: attention ============
        att_stash = []
        with tc.tile_pool(name="ps_sc", bufs=2, space="PSUM") as PSS, \
             tc.tile_pool(name="ps_ao", bufs=2, space="PSUM") as PSA, \
             tc.tile_pool(name="att_sb", bufs=3) as ASB, \
             tc.tile_pool(name="ao_sb", bufs=8) as AOSB:
            for lh in range(HPC):
                hq = qkT[64 * lh:64 * (lh + 1), 0:2048]
                hk = qkT[64 * lh:64 * (lh + 1), 2048:4096]
                for G in range(4):
                    nkb = 4 * G + 4
                    ao = PSA.tile([65, 512], F32, tag="ao")
                    for pj in range(nkb // 2):
                        ps = PSS.tile([128, 1024], F32, tag="ps_sc")
                        ex = ASB.tile([128, 1024], BF, tag="ex")
                        for half in range(2):
                            kb = 2 * pj + half
                            te.matmul(ps[:, 512 * half:512 * (half + 1)],
                                      hk[:, 128 * kb:128 * (kb + 1)],
                                      hq[:, 512 * G:512 * (G + 1)],
                                      start=True, stop=True)
                        s.activation(ex[:], ps[:], Act.Exp)
                        for half in range(2):
                            kb = 2 * pj + half
                            j = kb - 4 * G
                            exh = ex[:, 512 * half:512 * (half + 1)]
                            if 0 <= j < 4:
                                v.tensor_tensor(
                                    exh, exh, tri[:, 512 * j:512 * (j + 1)],
                                    Alu.mult)
                            te.matmul(
                                ao[:],
                                vaug[:, 137 * kb + 68 * lh:
                                     137 * kb + 68 * lh + 65],
                                exh,
                                start=(kb == 0), stop=(kb == nkb - 1))
                    aos = AOSB.tile([65, 512], BF, tag="aos")
                    s.copy(aos[:], ao[0:65, :])
                    dent = denpA if lh == 0 else denpB
                    v.tensor_copy(dent[32 * G:32 * G + 1, :], aos[64:65, :])
                    att_stash.append((lh, G, aos))
            v.reciprocal(denpA[:], denpA[:])
            v.reciprocal(denpB[:], denpB[:])
            for lh, G, aos in att_stash:
                rrow = ASB.tile([1, 512], BF, tag="rrow")
                dent = denpA if lh == 0 else denpB
                v.tensor_copy(rrow[0:1, :], dent[32 * G:32 * G + 1, :])
                rbp = PSA.tile([64, 512], F32, tag="rbp")
                te.matmul(rbp[:], onesr[:], rrow[:], start=True, stop=True)
                v.tensor_tensor(
                    aosc[64 * lh:64 * (lh + 1), 512 * G:512 * (G + 1)],
                    aos[0:64, :], rbp[:], Alu.mult)

        # ============ Phase 4: AllToAll ============
        for r in range(NCORES):
            nc.sync.dma_start(out=a2a_in.ap()[128 * r:128 * (r + 1), :],
                              in_=aosc[:, TOK * r:TOK * (r + 1)])
        g.collective_compute("AllToAll", Alu.bypass, replica_groups=RG,
                             ins=[a2a_in.ap()], outs=[a2a_out.ap()])
        for r in range(NCORES):
            nc.sync.dma_start(out=aofull[:, TOK * r:TOK * (r + 1)],
                              in_=a2a_out.ap()[128 * r:128 * (r + 1), :])
        # ============ Phase 5: out proj + norm1 ============
        with tc.tile_pool(name="ps_out", bufs=3, space="PSUM") as PSO, \
             tc.tile_pool(name="ps_r2", bufs=1, space="PSUM") as PSR2, \
             tc.tile_pool(name="tmp2", bufs=2) as TMP2:
            for of in range(8):
                ps = PSO.tile([128, TOK], F32, tag="ps_out")
                for kc in range(8):
                    te.matmul(
                        ps[:],
                        wout_s[:, (of * 8 + kc) * 128:(of * 8 + kc + 1) * 128],
                        aofull[:, TOK * kc:TOK * (kc + 1)],
                        start=(kc == 0), stop=(kc == 7))
                v.scalar_tensor_tensor(xt[:, TOK * of:TOK * (of + 1)],
                                       ps[:], blc("b_out", of),
                                       xt[:, TOK * of:TOK * (of + 1)],
                                       Alu.add, Alu.add)
            _ln_full(nc, tc, TMP2, PSR2, rows, xt, x1f, x1b8, ones32,
                     blob, "n1w", "n1b", "n1w16", "n1b16", cst[0:1, 0:1])
        WO_cm.__exit__(None, None, None)

        # ============ Phase 6: ff1 (fp8) + ep path + spline rows ============
        with tc.tile_pool(name="ps_h", bufs=2, space="PSUM") as PSH, \
             tc.tile_pool(name="ps_r3", bufs=1, space="PSUM") as PSR3, \
             tc.tile_pool(name="tmp3b", bufs=1) as T3B:
            t_sh = PSR3.tile([1, TOK], F32, tag="shp")
            t_sh2 = PSR3.tile([1, TOK], F32, tag="sh2p")
            t_se1 = PSR3.tile([1, TOK], F32, tag="se1p")
            t_se2 = PSR3.tile([1, TOK], F32, tag="se2p")
            t_pse2 = PSR3.tile([1, TOK], F32, tag="pse2p")
            sh, sh2, se1, se2, pse2 = (t_sh[:], t_sh2[:], t_se1[:],
                                       t_se2[:], t_pse2[:])
            hsqp = T3B.tile([128, TOK], BF, tag="hsqp")
            for c in range(32):
                ps = PSH.tile([128, TOK], F32, tag="ps_h")
                for kp in range(4):
                    te.matmul(ps[:],
                              wff1_s[:, c * 8 + 2 * kp:c * 8 + 2 * kp + 2, :],
                              x1b8[:, 2 * kp:2 * kp + 2, :],
                              start=(kp == 0), stop=(kp == 3),
                              perf_mode=PM.DoubleRow)
                hs = hb[:, TOK * c:TOK * (c + 1)]
                s.activation(hs, ps[:], Act.Identity,
                             bias=blc("b_ff1", c), scale=1.0 / (SW1 * SX1))
                g.tensor_tensor(hsqp[:], hs, hs, Alu.mult)
                te.matmul(sh, onesb[:], hs, start=(c == 0), stop=(c == 31))
                te.matmul(sh2, onesb[:], hsqp[:], start=(c == 0), stop=(c == 31))
            # ep path
            wep2_s = T3B.tile([128, 2], BF, tag="wep2_s")
            nc.sync.dma_start(out=wep2_s[:], in_=tin["wep2"].ap())
            epb = T3B.tile([128, 2 * TOK], BF, tag="epb")
            epsq = T3B.tile([128, TOK], BF, tag="epsq")
            for of in range(2):
                ps = PSH.tile([128, TOK], F32, tag="ps_h")
                for kc in range(32):
                    te.matmul(
                        ps[:],
                        wep1_s[:, (of * 32 + kc) * 128:(of * 32 + kc + 1) * 128],
                        hb[:, TOK * kc:TOK * (kc + 1)],
                        start=(kc == 0), stop=(kc == 31))
                s.activation(epb[:, TOK * of:TOK * (of + 1)], ps[:],
                             Act.Identity, bias=blc("b_ep1", of))
                v.tensor_tensor(epsq[:], epb[:, TOK * of:TOK * (of + 1)],
                                epb[:, TOK * of:TOK * (of + 1)], Alu.mult)
                te.matmul(se1, onesb[:], epb[:, TOK * of:TOK * (of + 1)],
                          start=(of == 0), stop=(of == 1))
                te.matmul(se2, onesb[:], epsq[:],
                          start=(of == 0), stop=(of == 1))
            _ln_rows(nc, rs(3), rs(4), rs(5), se1, se2, D16, cst[0:1, 0:1])
            mue_b = T3B.tile([128, TOK], F32, tag="mue_b")
            see_b = T3B.tile([128, TOK], F32, tag="see_b")
            g.partition_broadcast(mue_b[:], rs(3))
            g.partition_broadcast(see_b[:], rs(4))
            relub = T3B.tile([128, 2 * TOK], BF, tag="relub")
            tm3 = T3B.tile([128, TOK], F32, tag="tm3")
            for of in range(2):
                v.tensor_tensor(tm3[:], epb[:, TOK * of:TOK * (of + 1)],
                                mue_b[:], Alu.subtract)
                v.tensor_tensor(tm3[:], tm3[:], see_b[:], Alu.mult)
                v.tensor_scalar(tm3[:], tm3[:], blc("eplw", of),
                                blc("eplb", of), Alu.mult, Alu.add)
                v.tensor_scalar(relub[:, TOK * of:TOK * (of + 1)], tm3[:],
                                0.0, None, Alu.max)
            for of in range(2):
                te.matmul(pse2, wep2_s[:, of:of + 1],
                          relub[:, TOK * of:TOK * (of + 1)],
                          start=(of == 0), stop=(of == 1))
            erow = rs(6)
            s.activation(erow, pse2, Act.Sigmoid, bias=cst[0:1, 2:3])
            # emrep = SA * (1 + 0.1 e)
            v.tensor_scalar(erow, erow, 0.1 * SA, SA, Alu.mult, Alu.add)

            # spline per-token rows: mu_h (7), S (8)
            _spline_rows(nc, rs, sh, sh2, cst[0:1, 0:1])

            v.tensor_scalar(rs(0), rs(6), c0, None, Alu.mult)
            v.tensor_scalar(rs(1), rs(0), -64.0, SQ, Alu.mult, Alu.add)
            v.tensor_scalar(rs(2), rs(6), 64.0, None, Alu.mult)
            muh_b = T3B.tile([128, TOK], F32, tag="muh_b")
            Sh_b = T3B.tile([128, TOK], F32, tag="Sh_b")
            em_b = T3B.tile([128, TOK], F32, tag="em_b")
            thr_b = T3B.tile([128, TOK], F32, tag="thr_b")
            g.partition_broadcast(muh_b[:], rs(7))
            g.partition_broadcast(Sh_b[:], rs(8))
            g.partition_broadcast(em_b[:], rs(2))
            g.partition_broadcast(thr_b[:], rs(1))
            for (src, dst) in ((muh_b, murep), (Sh_b, Srep), (em_b, emqrep),
                               (thr_b, thrrep)):
                v.tensor_copy(dst[:], src[:].unsqueeze(1).to_broadcast((128, 8, TOK)))
        W6_cm.__exit__(None, None, None)
        # ============ Phase 7: spline + ff2 interleaved ============
        W7_cm = tc.tile_pool(name="w7_pool", bufs=1)
        W7 = W7_cm.__enter__()
        wff2_s = W7.tile([128, 256, 128], FP8, tag="wff2_s")
        for q in range(4):
            pq[q](out=wff2_s[:, 64 * q:64 * (q + 1), :],
                  in_=tin["wff2"].ap()[:, 8192 * q:8192 * (q + 1)])
        actt8 = W7.tile([128, 32, TOK], FP8, tag="actt8")
        with tc.tile_pool(name="spl", bufs=2) as SPL, \
             tc.tile_pool(name="ps_f2", bufs=1, space="PSUM") as PSF:
            accs = [PSF.tile([128, TOK], F32, tag=f"acc{of}", name=f"acc{of}")
                    for of in range(8)]
            for gi in range(4):
                hbs = hb[:, 2048 * gi:2048 * (gi + 1)]
                u = SPL.tile([128, 2048], BF, tag="u")
                t = SPL.tile([128, 2048], BF, tag="t")
                acc = SPL.tile([128, 2048], BF, tag="acc")
                g.tensor_tensor(u[:], hbs, murep[:], Alu.subtract)
                g.tensor_tensor(u[:], u[:], Srep[:], Alu.mult)
                v.scalar_tensor_tensor(t[:], u[:], -1.0, u[:],
                                       Alu.mult, Alu.max)
                v.tensor_scalar(acc[:], t[:], c2, c3, Alu.mult, Alu.add)
                v.tensor_tensor(acc[:], acc[:], t[:], Alu.mult)
                v.scalar_tensor_tensor(acc[:], u[:], c1, acc[:],
                                       Alu.mult, Alu.add)
                v.tensor_tensor(acc[:], acc[:], emqrep[:], Alu.mult)
                v.tensor_tensor(
                    actt8[:, 8 * gi:8 * (gi + 1), :],
                    acc[:], thrrep[:], Alu.min)
                for of in range(8):
                    for kp in range(4):
                        kk = 8 * gi + 2 * kp
                        te.matmul(accs[of][:],
                                  wff2_s[:, of * 32 + kk:of * 32 + kk + 2, :],
                                  actt8[:, kk:kk + 2, :],
                                  start=(gi == 0 and kp == 0),
                                  stop=(gi == 3 and kp == 3),
                                  perf_mode=PM.DoubleRow)
            em0c = SPL.tile([128, TOK], F32, tag="em0c", bufs=1)
            g.partition_broadcast(em0c[:], rs(0))
            for of in range(8):
                v.tensor_scalar(x1f[:, TOK * of:TOK * (of + 1)],
                                x1f[:, TOK * of:TOK * (of + 1)],
                                blc("b_ff2", of), None, Alu.add)
                v.scalar_tensor_tensor(x1f[:, TOK * of:TOK * (of + 1)],
                                       accs[of][:], 1.0 / (SQ * SW2),
                                       x1f[:, TOK * of:TOK * (of + 1)],
                                       Alu.mult, Alu.add)
                v.scalar_tensor_tensor(x1f[:, TOK * of:TOK * (of + 1)],
                                       em0c[:], blc("rsw2", of),
                                       x1f[:, TOK * of:TOK * (of + 1)],
                                       Alu.mult, Alu.add)
        W7_cm.__exit__(None, None, None)
        with tc.tile_pool(name="ps_r4", bufs=1, space="PSUM") as PSR4, \
             tc.tile_pool(name="tmp4", bufs=2) as TMP4:
            _ln_full(nc, tc, TMP4, PSR4, rows, x1f, x1f, None, ones32,
                     blob, "n2w", "n2b", None, None, cst[0:1, 0:1])
        TMP3_cm.__exit__(None, None, None)
        nc.sync.dma_start(out=t_out.ap(), in_=x1f[:])


def _spline_rows(nc, rs, sh, sh2, epsap):
    """rs(7) = mu_h, rs(8) = S = 1/(sqrt(var+eps)*(norm+1)),
    norm = sqrt(FF*var/(var+eps) + eps)."""
    v, s = nc.vector, nc.scalar
    mu = rs(7)
    S_ = rs(8)
    var = rs(13)
    t1 = rs(14)
    t2 = rs(15)
    v.tensor_scalar(mu, sh, 1.0 / FF, None, Alu.mult)
    v.tensor_tensor(var, mu, mu, Alu.mult)
    v.tensor_scalar(t1, sh2, 1.0 / FF, None, Alu.mult)
    v.tensor_tensor(var, t1, var, Alu.subtract)
    s.activation(t1, var, Act.Ln, bias=epsap)
    s.activation(t1, t1, Act.Exp, scale=0.5)          # sqrt(var+eps)
    v.tensor_scalar(t2, var, EPS, None, Alu.add)
    v.reciprocal(t2, t2)
    v.tensor_tensor(t2, t2, var, Alu.mult)
    v.tensor_scalar(t2, t2, float(FF), None, Alu.mult)
    s.activation(t2, t2, Act.Ln, bias=epsap)
    s.activation(t2, t2, Act.Exp, scale=0.5)          # norm
    v.tensor_scalar(t2, t2, 1.0, None, Alu.add)
    v.tensor_tensor(t2, t2, t1, Alu.mult)
    v.reciprocal(S_, t2)


def _ln_full(nc, tc, TMP, PSR, rows, src, dstf, dst8, ones32, blob,
             wnm, bnm, w16nm, b16nm, epsap):
    co = lambda nm, k: blob[:, _CO[nm] + k:_CO[nm] + k + 1]
    v, s, g, te = nc.vector, nc.scalar, nc.gpsimd, nc.tensor
    T = TOK
    rs = lambda k: rows[0:1, k * T:(k + 1) * T]
    t_sx = PSR.tile([1, T], F32, tag="lnsxp")
    t_sx2 = PSR.tile([1, T], F32, tag="lnsx2p")
    sx, sx2 = t_sx[:], t_sx2[:]
    for kc in range(8):
        te.matmul(sx, ones32[:], src[:, T * kc:T * (kc + 1)],
                  start=(kc == 0), stop=(kc == 7))
    xsq = TMP.tile([128, T], F32, tag="lnxsq")
    for kc in range(8):
        v.tensor_tensor(xsq[:], src[:, T * kc:T * (kc + 1)],
                        src[:, T * kc:T * (kc + 1)], Alu.mult)
        te.matmul(sx2, ones32[:], xsq[:], start=(kc == 0), stop=(kc == 7))
    _ln_rows(nc, rs(9), rs(10), rs(11), sx, sx2, D, epsap)
    mu_b = TMP.tile([128, T], F32, tag="lnmu_b")
    s_b = TMP.tile([128, T], F32, tag="lns_b")
    g.partition_broadcast(mu_b[:], rs(9))
    g.partition_broadcast(s_b[:], rs(10))
    tm = TMP.tile([128, T], F32, tag="lntm")
    for kc in range(8):
        v.tensor_tensor(tm[:], src[:, T * kc:T * (kc + 1)], mu_b[:],
                        Alu.subtract)
        v.tensor_tensor(tm[:], tm[:], s_b[:], Alu.mult)
        v.tensor_scalar(dstf[:, T * kc:T * (kc + 1)], tm[:],
                        co(wnm, kc), co(bnm, kc),
                        Alu.mult, Alu.add)
        if dst8 is not None:
            s.activation(dst8[:, kc:kc + 1, :], tm[:], Act.Identity,
                         bias=co(b16nm, kc), scale=co(w16nm, kc))


# ----------------------------------------------------------------------------
# Entry point
# ----------------------------------------------------------------------------

def kernel(**inputs):
    in_maps, sc = _prepare_inputs(inputs)
    key = hashlib.sha256(
        repr((sc["coeffs"], sc["ent_b"], sc["ep2_b"])).encode()
    ).hexdigest()
    if key not in _prog_cache:
        _prog_cache[key] = _build_program(sc)
    nc = _prog_cache[key]
    res = bass_utils.run_bass_kernel_spmd(nc, in_maps,
                                          core_ids=list(range(NCORES)))
    out = np.empty((1, S, D), np.float32)
    for c in range(NCORES):
        oc = np.asarray(res.results[c]["out"], np.float32)   # [128, 8*TOK]
        ot = oc.reshape(128, 8, TOK).transpose(1, 0, 2).reshape(D, TOK)
        out[0, c * TOK:(c + 1) * TOK, :] = ot.T
    return out


# revision 22
# speedup vs baseline: 1.2528x; 1.2528x over previous
"""Trainium2 8-core kernel for nn_EnhancedTransformerBlock (v2).

SPMD: identical program on all 8 cores.
  - Full x replicated to every core (bf16) -> no AllGather. Each core
    computes LN stats for all 2048 tokens, normalizes x, then QKV for its
    2 heads (head-sharded attention over the full sequence).
  - ln_attn affine and softmax temperature folded into QKV weights host-side.
  - Entropy gate folded into the V GEMM as an extra output column.
  - Attention: unshifted exp, denominator via ones-column on V, causal
    triangle masks, single reciprocal for all 8 (head, q-group) denominators.
  - AllToAll of per-head attention outputs back to sequence sharding
    (core c owns tokens [256c, 256c+256) for the FFN part).
  - ff1/ff2 GEMMs in fp8(e4m3) DoubleRow mode (K=256 per instruction,
    2x rate); activations scaled x16/x64 to sit in fp8's normal range.
  - Spline activation approximated by a 4-term kink-basis LSQ fit of the
    fixed 1-D function g(u) (computed host-side from runtime knots/spl_w);
    evaluated in ~9 elementwise ops per 2048-col group, alternating
    Vector/GpSimd engines; ff2 partial GEMMs interleaved with spline groups.
"""

import hashlib
import numpy as np

from concourse import bacc, tile, mybir
from concourse import bass_utils

dt = mybir.dt
BF = dt.bfloat16
F32 = dt.float32
FP8 = dt.float8e4
NPBF = dt.np(BF)
NPF8 = dt.np(FP8)
Alu = mybir.AluOpType
Act = mybir.ActivationFunctionType
PM = mybir.MatmulPerfMode

NCORES = 8
S = 2048
D = 1024
H = 16
HD = 64
FF = 4096
D16 = 256
TOK = S // NCORES            # 256 tokens per core
HPC = H // NCORES            # 2 heads per core
EPS = 1e-6
UDOM = 0.12                  # spline fit domain |u| <= UDOM (|u| < 0.09 true)

SX1 = 16.0                   # x1 fp8 scale
SW1 = 64.0                   # ff1_w fp8 scale
SA = 64.0                    # act fp8 scale
SW2 = 64.0                   # ff2_w fp8 scale
SQ = 4096.0                  # spline-delta fp8 scale

_prog_cache = {}


# ----------------------------------------------------------------------------
# Host-side: spline fit
# ----------------------------------------------------------------------------

def _g_exact(u, knots, spl_w):
    d = np.abs(u[:, None] - knots[None, :])
    d = d / (d.max(-1, keepdims=True) + EPS)
    a = -5.0 * d
    a = a - a.max(-1, keepdims=True)
    e = np.exp(a)
    p = e / e.sum(-1, keepdims=True)
    return (p * spl_w).sum(-1)


def _fit_spline(knots, spl_w):
    """LSQ fit of g(u) on [-UDOM, UDOM]; basis [1, u, u^2, |u|]."""
    k = np.asarray(knots, np.float64)
    w = np.asarray(spl_w, np.float64)
    u = np.linspace(-UDOM, UDOM, 20001)
    B = np.stack([np.ones_like(u), u, u * u, np.abs(u)], -1)
    y = _g_exact(u, k, w)
    c, *_ = np.linalg.lstsq(B, y, rcond=None)
    err = float(np.abs(B @ c - y).max())
    return [float(v) for v in c], err


# ----------------------------------------------------------------------------
# Host-side: weight packing
# ----------------------------------------------------------------------------

def _pack_lhsT(w_t, n_of, n_kc):
    """w_t: [K_total, M_total] ([in, out]) -> [128, n_of*n_kc*128], tile
    (of, kc) at cols [(of*n_kc+kc)*128 ...] = w_t[128kc:.., 128of:..]."""
    K_total, M_total = w_t.shape
    assert K_total == n_kc * 128 and M_total == n_of * 128
    out = np.empty((128, n_of * n_kc * 128), np.float32)
    for of in range(n_of):
        for kc in range(n_kc):
            out[:, (of * n_kc + kc) * 128:(of * n_kc + kc + 1) * 128] = \
                w_t[kc * 128:(kc + 1) * 128, of * 128:(of + 1) * 128]
    return np.ascontiguousarray(out)


def _col_pack(vec, n_chunks):
    return np.ascontiguousarray(
        np.asarray(vec, np.float32).reshape(n_chunks, 128).T)


def _make_tri_masks():
    out = np.zeros((128, 4 * 512), np.float32)
    for j in range(4):
        kk = np.arange(128)[:, None] + 128 * j
        q = np.arange(512)[None, :]
        out[:, 512 * j:512 * (j + 1)] = (kk <= q).astype(np.float32)
    return out


# const blob layout (f32 [128, CW]); col offsets
_CO = {}
_cw = 0
def _co(name, w):
    global _cw
    _CO[name] = _cw
    _cw += w
_co("b_qk", 2)
_co("b_out", 8)
_co("b_ff1", 32)
_co("b_ff2", 8)
_co("b_ep1", 2)
_co("n1w", 8)
_co("n1b", 8)
_co("n1w16", 8)
_co("n1b16", 8)
_co("n2w", 8)
_co("n2b", 8)
_co("eplw", 2)
_co("eplb", 2)
_co("rsw2", 8)
_co("bv_row", 137)           # row 0 only
CW = _cw


def _prepare_inputs(inputs):
    f = lambda k: np.asarray(inputs[k], np.float32)
    x = f("x").reshape(S, D)
    qkv_w, qkv_b = f("qkv_w"), f("qkv_b")
    out_w, out_b = f("out_w") * 0.1, f("out_b") * 0.1
    ff1_w, ff1_b = f("ff1_w"), f("ff1_b")
    ff2_w, ff2_b = f("ff2_w"), f("ff2_b")
    ep1_w, ep1_b = f("ep1_w"), f("ep1_b")
    ep2_w, ep2_b = f("ep2_w"), f("ep2_b")
    ent_w, ent_b = f("ent_w"), f("ent_b")
    lnw, lnb = f("ln_attn_w"), f("ln_attn_b")
    n1w, n1b = f("norm1_w"), f("norm1_b")

    temp = (1.0 / np.sqrt(np.float32(HD))) / 0.1   # 1.25
    # fold ln_attn affine into qkv/ent weights: W'(xn) + b' == W(xl) + b
    wq = qkv_w[0:D] * temp * lnw[None, :]
    wk = qkv_w[D:2 * D] * lnw[None, :]
    wv = qkv_w[2 * D:3 * D] * lnw[None, :]
    bq = qkv_b[0:D] * temp + wq @ lnb
    bk = qkv_b[D:2 * D] + wk @ lnb
    bv = qkv_b[2 * D:3 * D] + wv @ lnb
    went = ent_w.reshape(D) * lnw
    bent = float(ent_b.reshape(-1)[0] + went @ lnb)

    coeffs, fit_err = _fit_spline(f("knots"), f("spl_w"))

    xT = np.ascontiguousarray(x.T)                           # [D, S]
    xfull = np.ascontiguousarray(
        xT.reshape(8, 128, S).transpose(1, 0, 2).reshape(128, 8 * S)
    ).astype(NPBF)

    shared = {
        "xfull": xfull,
        "tri": _make_tri_masks().astype(NPBF),
        "wout": _pack_lhsT(out_w.T, 8, 8).astype(NPBF),
        "wff1": np.ascontiguousarray(
            _pack_lhsT(ff1_w.T * SW1, 32, 8)).astype(NPF8),
        "wff2": np.ascontiguousarray(
            _pack_lhsT(ff2_w.T * SW2, 8, 32)).astype(NPF8),
        "wep1": _pack_lhsT(ep1_w.T, 2, 32).astype(NPBF),
        "wep2": np.ascontiguousarray(
            ep2_w.reshape(2, 128).T).astype(NPBF),          # [128, 2]
    }

    scalars = {
        "ent_b": bent,
        "ep2_b": float(ep2_b.reshape(-1)[0]),
        "coeffs": coeffs,
        "fit_err": fit_err,
    }

    in_maps = []
    for c in range(NCORES):
        m = dict(shared)
        xc = x[c * TOK:(c + 1) * TOK]                        # [256, D]
        xTc = np.ascontiguousarray(xc.T)                     # [D, 256]
        m["xT"] = np.ascontiguousarray(
            xTc.reshape(8, 128, TOK).transpose(1, 0, 2).reshape(128, 8 * TOK))
        h0 = c * HPC
        wq_c = wq[h0 * HD:(h0 + HPC) * HD]                   # [128, D]
        wk_c = wk[h0 * HD:(h0 + HPC) * HD]
        wqk_t = np.concatenate([wq_c, wk_c], 0).T            # [D, 256]
        m["wqk"] = _pack_lhsT(wqk_t, 2, 8).astype(NPBF)
        # V weights [D, 137]: v_h0(64) pad(4) v_h1(64) pad(4) went(1)
        wv_c = wv[h0 * HD:(h0 + HPC) * HD].T                 # [D, 128]
        wva = np.zeros((D, 137), np.float32)
        bva = np.zeros((1, 137), np.float32)
        for lh in range(HPC):
            wva[:, 68 * lh:68 * lh + 64] = wv_c[:, 64 * lh:64 * lh + 64]
            bva[0, 68 * lh:68 * lh + 64] = \
                bv[(h0 + lh) * HD:(h0 + lh + 1) * HD]
        wva[:, 136] = went
        m["wv"] = np.ascontiguousarray(
            wva.reshape(8, 128, 137).transpose(1, 0, 2).reshape(128, 8 * 137)
        ).astype(NPBF)
        # const blob
        blob = np.zeros((128, CW), np.float32)
        def put(name, arr):
            a = np.asarray(arr, np.float32)
            blob[:, _CO[name]:_CO[name] + a.shape[1]] = a
        put("b_qk", np.stack([bq[h0 * HD:(h0 + HPC) * HD],
                              bk[h0 * HD:(h0 + HPC) * HD]], -1))
        put("b_out", _col_pack(out_b, 8))
        put("b_ff1", _col_pack(ff1_b, 32))
        put("b_ff2", _col_pack(ff2_b, 8))
        put("b_ep1", _col_pack(ep1_b, 2))
        put("n1w", _col_pack(n1w, 8))
        put("n1b", _col_pack(n1b, 8))
        put("n1w16", _col_pack(n1w * SX1, 8))
        put("n1b16", _col_pack(n1b * SX1, 8))
        put("n2w", _col_pack(f("norm2_w"), 8))
        put("n2b", _col_pack(f("norm2_b"), 8))
        put("eplw", _col_pack(f("ep_ln_w"), 2))
        put("eplb", _col_pack(f("ep_ln_b"), 2))
        put("rsw2", _col_pack(ff2_w.sum(1) / SA, 8))
        blob[0, _CO["bv_row"]:_CO["bv_row"] + 137] = bva[0]
        m["blob"] = np.ascontiguousarray(blob)
        in_maps.append(m)

    return in_maps, scalars


# ----------------------------------------------------------------------------
# Device program
# ----------------------------------------------------------------------------

def _build_program(sc):
    nc = bacc.Bacc("TRN2", target_bir_lowering=False, debug=False,
                   num_devices=NCORES)

    def din(name, shape, dtype):
        return nc.dram_tensor(name, list(shape), dtype, kind="ExternalInput")

    tin = {
        "xfull": din("xfull", (128, 8 * S), BF),
        "xT": din("xT", (128, 8 * TOK), F32),
        "wqk": din("wqk", (128, 2048), BF),
        "wv": din("wv", (128, 8 * 137), BF),
        "wout": din("wout", (128, 8192), BF),
        "wff1": din("wff1", (128, 32768), FP8),
        "wff2": din("wff2", (128, 32768), FP8),
        "wep1": din("wep1", (128, 8192), BF),
        "wep2": din("wep2", (128, 2), BF),
        "tri": din("tri", (128, 2048), BF),
        "blob": din("blob", (128, CW), F32),
    }
    t_out = nc.dram_tensor("out", [128, 8 * TOK], F32, kind="ExternalOutput")
    a2a_in = nc.dram_tensor("a2a_in", [1024, TOK], BF, kind="Internal")
    a2a_out = nc.dram_tensor("a2a_out", [1024, TOK], BF, kind="Internal")

    with tile.TileContext(nc) as tc:
        _emit(nc, tc, tin, t_out, a2a_in, a2a_out, sc)
    nc.compile()
    return nc


def _ln_rows(nc, mu, st, tmp, sx, sx2, n, epsap):
    """mu = sx/n; st = 1/sqrt(var+eps) with var = sx2/n - mu^2."""
    v, s = nc.vector, nc.scalar
    v.tensor_scalar(mu, sx, 1.0 / n, None, Alu.mult)
    v.tensor_tensor(st, mu, mu, Alu.mult)
    v.tensor_scalar(tmp, sx2, 1.0 / n, None, Alu.mult)
    v.tensor_tensor(st, tmp, st, Alu.subtract)
    s.activation(st, st, Act.Ln, bias=epsap)
    s.activation(st, st, Act.Exp, scale=-0.5)


def _emit(nc, tc, tin, t_out, a2a_in, a2a_out, sc):
    v = nc.vector
    s = nc.scalar
    g = nc.gpsimd
    te = nc.tensor
    c0, c1, c2, c3 = sc["coeffs"]
    # q(u) = c1 u + c2 u^2 + c3|u| == sgn2*Square(sq_s*u + sq_b)
    #        + sgn3*Abs(|c3| u) - sgn2*sq_b^2
    sgn2 = 1.0 if c2 >= 0 else -1.0
    sgn3 = 1.0 if c3 >= 0 else -1.0
    sq_s = float(np.sqrt(max(abs(c2), 1e-6)))
    sq_b = c1 / (2.0 * sq_s * sgn2)
    c0eff = c0 - sgn2 * sq_b * sq_b
    RG = [list(range(NCORES))]

    with tc.tile_pool(name="persist", bufs=1) as P, \
         tc.tile_pool(name="consts", bufs=1) as C, \
         tc.tile_pool(name="rows", bufs=1) as R:

        # ---- constants + input DMAs, spread across queues ----
        blob = C.tile([128, CW], F32, tag="blob")
        nc.sync.dma_start(out=blob[:], in_=tin["blob"].ap())
        blc = lambda nm, k: blob[:, _CO[nm] + k:_CO[nm] + k + 1]

        tri = C.tile([128, 2048], BF, tag="tri")
        g.dma_start(out=tri[:], in_=tin["tri"].ap())
        xt = P.tile([128, 8 * TOK], F32, tag="xt")
        g.dma_start(out=xt[:], in_=tin["xT"].ap())

        XN_cm = tc.tile_pool(name="xn_pool", bufs=1)
        XN = XN_cm.__enter__()
        xn = XN.tile([128, 8 * S], BF, tag="xn")
        XF_cm = tc.tile_pool(name="xf_pool", bufs=1)
        XF = XF_cm.__enter__()
        xf = XF.tile([128, 8 * S], BF, tag="xf")
        qdma = [nc.sync.dma_start, nc.scalar.dma_start,
                nc.gpsimd.dma_start, nc.sync.dma_start]
        for q in range(4):
            qdma[q](out=xf[:, 4096 * q:4096 * (q + 1)],
                    in_=tin["xfull"].ap()[:, 4096 * q:4096 * (q + 1)])

        onesb = C.tile([128, 1], BF, tag="onesb")
        ones32 = C.tile([128, 1], F32, tag="ones32")
        onesr = C.tile([1, 64], BF, tag="onesr")
        v.memset(onesb[:], 1.0)
        v.memset(ones32[:], 1.0)
        v.memset(onesr[:], 1.0)
        cst = C.tile([128, 4], F32, tag="cst")
        v.memset(cst[:, 0:1], EPS)
        v.memset(cst[:, 1:2], sc["ent_b"])
        v.memset(cst[:, 2:3], sc["ep2_b"])
        v.memset(cst[:, 3:4], sgn2 * sq_b)
        bvb = C.tile([128, 137], F32, tag="bvb")
        g.partition_broadcast(bvb[:], blob[0:1, _CO["bv_row"]:_CO["bv_row"] + 137])

        # persistent activations
        qkT = P.tile([128, 4096], BF, tag="qkT")
        vaug = P.tile([128, 16 * 137], BF, tag="vaug")
        aosc = P.tile([128, 2048], BF, tag="aosc")
        aofull = P.tile([128, 8 * TOK], BF, tag="aofull")
        x1f = P.tile([128, 8 * TOK], F32, tag="x1f")
        x1b8 = P.tile([128, 8, TOK], FP8, tag="x1b8")
        rows = R.tile([1, 16 * TOK], F32, tag="rows")
        rs = lambda k: rows[0:1, k * TOK:(k + 1) * TOK]
        denpA = R.tile([128, 512], F32, tag="denpA")
        denpB = R.tile([128, 512], F32, tag="denpB")

        # ============ Phase 1: full-seq LN stats + normalize ============
        with tc.tile_pool(name="ps_r1", bufs=4, space="PSUM") as PSR, \
             tc.tile_pool(name="tmp1", bufs=1) as TMP:
            mu_r = TMP.tile([1, S], F32, tag="mu_r")
            st_r = TMP.tile([1, S], F32, tag="st_r")
            tm_r = TMP.tile([1, S], F32, tag="tm_r")
            for w in range(4):
                sx = PSR.tile([1, 512], F32, tag="sx1p")
                sx2 = PSR.tile([1, 512], F32, tag="sx2p")
                sx, sx2 = sx[:], sx2[:]
                for kc in range(8):
                    te.matmul(sx, onesb[:],
                              xf[:, 2048 * kc + 512 * w:2048 * kc + 512 * (w + 1)],
                              start=(kc == 0), stop=(kc == 7))
                for kc in range(8):
                    xsq = TMP.tile([128, 512], BF, tag="xsq", bufs=4)
                    eng = v if kc % 2 == 0 else g
                    eng.tensor_tensor(
                        xsq[:], xf[:, 2048 * kc + 512 * w:2048 * kc + 512 * (w + 1)],
                        xf[:, 2048 * kc + 512 * w:2048 * kc + 512 * (w + 1)],
                        Alu.mult)
                    te.matmul(sx2, onesb[:], xsq[:],
                              start=(kc == 0), stop=(kc == 7))
                v.tensor_scalar(mu_r[0:1, 512 * w:512 * (w + 1)], sx,
                                1.0 / D, None, Alu.mult)
                v.tensor_scalar(st_r[0:1, 512 * w:512 * (w + 1)], sx2,
                                1.0 / D, None, Alu.mult)
            v.tensor_tensor(tm_r[:], mu_r[:], mu_r[:], Alu.mult)
            v.tensor_tensor(st_r[:], st_r[:], tm_r[:], Alu.subtract)
            s.activation(st_r[:], st_r[:], Act.Ln, bias=cst[0:1, 0:1])
            s.activation(st_r[:], st_r[:], Act.Exp, scale=-0.5)
            v.tensor_tensor(tm_r[:], mu_r[:], st_r[:], Alu.mult)
            # bf16 reps
            str_b = TMP.tile([1, S], BF, tag="str_b")
            mst_b = TMP.tile([1, S], BF, tag="mst_b")
            v.tensor_copy(str_b[:], st_r[:])
            v.tensor_copy(mst_b[:], tm_r[:])
            strep = TMP.tile([128, S], BF, tag="strep")
            mstrep = TMP.tile([128, S], BF, tag="mstrep")
            g.partition_broadcast(strep[:], str_b[:])
            g.partition_broadcast(mstrep[:], mst_b[:])
            for kc in range(8):
                tm = TMP.tile([128, S], BF, tag="nrm", bufs=2)
                eng = v if kc % 2 == 0 else g
                eng.tensor_tensor(tm[:], xf[:, 2048 * kc:2048 * (kc + 1)],
                                  strep[:], Alu.mult)
                eng.tensor_tensor(xn[:, 2048 * kc:2048 * (kc + 1)],
                                  tm[:], mstrep[:], Alu.subtract)
        XF_cm.__exit__(None, None, None)

        # ============ Phase 2: QKV + ent ============
        with tc.tile_pool(name="wq_pool", bufs=1) as WQ, \
             tc.tile_pool(name="ps_qk", bufs=2, space="PSUM") as PSQ, \
             tc.tile_pool(name="ps_ev", bufs=3, space="PSUM") as PSV, \
             tc.tile_pool(name="estmp", bufs=4) as EST:
            wv_s = WQ.tile([128, 8 * 137], BF, tag="wv_s")
            nc.scalar.dma_start(out=wv_s[:], in_=tin["wv"].ap())
            wqk_s = WQ.tile([128, 2048], BF, tag="wqk_s")
            nc.sync.dma_start(out=wqk_s[:], in_=tin["wqk"].ap())

            for tch in range(16):
                psv = PSV.tile([128, 137], F32, tag="psv", bufs=2)
                for kc in range(8):
                    te.matmul(
                        psv[:],
                        xn[:, 2048 * kc + 128 * tch:2048 * kc + 128 * (tch + 1)],
                        wv_s[:, 137 * kc:137 * (kc + 1)],
                        start=(kc == 0), stop=(kc == 7))
                esc = EST.tile([128, 1], F32, tag="esc")
                s.activation(esc[:], psv[:, 136:137], Act.Sigmoid,
                             bias=cst[:, 1:2])
                v.tensor_scalar(esc[:], esc[:], 0.1, None, Alu.max)
                vt = vaug[:, 137 * tch:137 * tch + 136]
                v.tensor_tensor(vt, psv[:, 0:136], bvb[:, 0:136], Alu.add)
                v.tensor_scalar(vt, vt, esc[:], None, Alu.mult)
                for lh in range(HPC):
                    v.memset(vaug[:, 137 * tch + 68 * lh + 64:
                                  137 * tch + 68 * lh + 65], 1.0)

            for of in range(2):
                for w in range(4):
                    ps = PSQ.tile([128, 512], F32, tag="psqk")
                    for kc in range(8):
                        te.matmul(
                            ps[:],
                            wqk_s[:, (of * 8 + kc) * 128:(of * 8 + kc + 1) * 128],
                            xn[:, 2048 * kc + 512 * w:2048 * kc + 512 * (w + 1)],
                            start=(kc == 0), stop=(kc == 7))
                    v.tensor_scalar(
                        qkT[:, 2048 * of + 512 * w:2048 * of + 512 * (w + 1)],
                        ps[:], blc("b_qk", of), None, Alu.add)
        XN_cm.__exit__(None, None, None)

        # ---- prefetch big weights (land during attention) ----
        TMP3_cm = tc.tile_pool(name="tmp3", bufs=1)
        TMP3 = TMP3_cm.__enter__()
        hb = TMP3.tile([128, 8192], BF, tag="hb")
        murep = TMP3.tile([128, 2048], BF, tag="murep")
        Srep = TMP3.tile([128, 2048], BF, tag="Srep")
        emqrep = TMP3.tile([128, 2048], BF, tag="emqrep")
        thrrep = TMP3.tile([128, 2048], BF, tag="thrrep")
        W6_cm = tc.tile_pool(name="w6_pool", bufs=1)
        W6 = W6_cm.__enter__()
        WO_cm = tc.tile_pool(name="wo_pool", bufs=1)
        WO = WO_cm.__enter__()
        wout_s = WO.tile([128, 8192], BF, tag="wout_s")
        wep1_s = W6.tile([128, 8192], BF, tag="wep1_s")
        wff1_s = W6.tile([128, 256, 128], FP8, tag="wff1_s")
        pq = [nc.sync.dma_start, nc.scalar.dma_start, nc.gpsimd.dma_start,
              nc.sync.dma_start]
        for q in range(4):
            pq[q](out=wout_s[:, 2048 * q:2048 * (q + 1)],
                  in_=tin["wout"].ap()[:, 2048 * q:2048 * (q + 1)])
        for q in range(4):
            pq[q](out=wff1_s[:, 64 * q:64 * (q + 1), :],
                  in_=tin["wff1"].ap()[:, 8192 * q:8192 * (q + 1)])
        for q in range(4):
            pq[q](out=wep1_s[:, 2048 * q:2048 * (q + 1)],
                  in_=tin["wep1"].ap()[:, 2048 * q:2048 * (q + 1)])

        # ============ Phase 3: attention ============
        att_stash = []
        with tc.tile_pool(name="ps_sc", bufs=2, space="PSUM") as PSS, \
             tc.tile_pool(name="ps_ao", bufs=2, space="PSUM") as PSA, \
             tc.tile_pool(name="att_sb", bufs=3) as ASB, \
             tc.tile_pool(name="ao_sb", bufs=8) as AOSB:
            for lh in range(HPC):
                hq = qkT[64 * lh:64 * (lh + 1), 0:2048]
                hk = qkT[64 * lh:64 * (lh + 1), 2048:4096]
                for G in range(4):
                    nkb = 4 * G + 4
                    ao = PSA.tile([65, 512], F32, tag="ao")
                    for pj in range(nkb // 2):
                        ps = PSS.tile([128, 1024], F32, tag="ps_sc")
                        ex = ASB.tile([128, 1024], BF, tag="ex")
                        for half in range(2):
                            kb = 2 * pj + half
                            te.matmul(ps[:, 512 * half:512 * (half + 1)],
                                      hk[:, 128 * kb:128 * (kb + 1)],
                                      hq[:, 512 * G:512 * (G + 1)],
                                      start=True, stop=True)
                        s.activation(ex[:], ps[:], Act.Exp)
                        for half in range(2):
                            kb = 2 * pj + half
                            j = kb - 4 * G
                            exh = ex[:, 512 * half:512 * (half + 1)]
                            if 0 <= j < 4:
                                v.tensor_tensor(
                                    exh, exh, tri[:, 512 * j:512 * (j + 1)],
                                    Alu.mult)
                            te.matmul(
                                ao[:],
                                vaug[:, 137 * kb + 68 * lh:
                                     137 * kb + 68 * lh + 65],
                                exh,
                                start=(kb == 0), stop=(kb == nkb - 1))
                    aos = AOSB.tile([65, 512], BF, tag="aos")
                    s.copy(aos[:], ao[0:65, :])
                    dent = denpA if lh == 0 else denpB
                    v.tensor_copy(dent[32 * G:32 * G + 1, :], aos[64:65, :])
                    att_stash.append((lh, G, aos))
            v.reciprocal(denpA[:], denpA[:])
            v.reciprocal(denpB[:], denpB[:])
            for lh, G, aos in att_stash:
                rrow = ASB.tile([1, 512], BF, tag="rrow")
                dent = denpA if lh == 0 else denpB
                v.tensor_copy(rrow[0:1, :], dent[32 * G:32 * G + 1, :])
                rbp = PSA.tile([64, 512], F32, tag="rbp")
                te.matmul(rbp[:], onesr[:], rrow[:], start=True, stop=True)
                v.tensor_tensor(
                    aosc[64 * lh:64 * (lh + 1), 512 * G:512 * (G + 1)],
                    aos[0:64, :], rbp[:], Alu.mult)

        # ============ Phase 4: AllToAll ============
        for r in range(NCORES):
            nc.sync.dma_start(out=a2a_in.ap()[128 * r:128 * (r + 1), :],
                              in_=aosc[:, TOK * r:TOK * (r + 1)])
        g.collective_compute("AllToAll", Alu.bypass, replica_groups=RG,
                             ins=[a2a_in.ap()], outs=[a2a_out.ap()])
        for r in range(NCORES):
            nc.sync.dma_start(out=aofull[:, TOK * r:TOK * (r + 1)],
                              in_=a2a_out.ap()[128 * r:128 * (r + 1), :])
        # ============ Phase 5: out proj + norm1 ============
        with tc.tile_pool(name="ps_out", bufs=3, space="PSUM") as PSO, \
             tc.tile_pool(name="ps_r2", bufs=1, space="PSUM") as PSR2, \
             tc.tile_pool(name="tmp2", bufs=2) as TMP2:
            for of in range(8):
                ps = PSO.tile([128, TOK], F32, tag="ps_out")
                for kc in range(8):
                    te.matmul(
                        ps[:],
                        wout_s[:, (of * 8 + kc) * 128:(of * 8 + kc + 1) * 128],
                        aofull[:, TOK * kc:TOK * (kc + 1)],
                        start=(kc == 0), stop=(kc == 7))
                v.scalar_tensor_tensor(xt[:, TOK * of:TOK * (of + 1)],
                                       ps[:], blc("b_out", of),
                                       xt[:, TOK * of:TOK * (of + 1)],
                                       Alu.add, Alu.add)
            _ln_full(nc, tc, TMP2, PSR2, rows, xt, x1f, x1b8, ones32,
                     blob, "n1w", "n1b", "n1w16", "n1b16", cst[0:1, 0:1])
        WO_cm.__exit__(None, None, None)

        # ============ Phase 6: ff1 (fp8) + ep path + spline rows ============
        with tc.tile_pool(name="ps_h", bufs=2, space="PSUM") as PSH, \
             tc.tile_pool(name="ps_r3", bufs=1, space="PSUM") as PSR3, \
             tc.tile_pool(name="tmp3b", bufs=1) as T3B:
            t_sh = PSR3.tile([1, TOK], F32, tag="shp")
            t_sh2 = PSR3.tile([1, TOK], F32, tag="sh2p")
            t_se1 = PSR3.tile([1, TOK], F32, tag="se1p")
            t_se2 = PSR3.tile([1, TOK], F32, tag="se2p")
            t_pse2 = PSR3.tile([1, TOK], F32, tag="pse2p")
            sh, sh2, se1, se2, pse2 = (t_sh[:], t_sh2[:], t_se1[:],
                                       t_se2[:], t_pse2[:])
            hsqp = T3B.tile([128, TOK], BF, tag="hsqp")
            for c in range(32):
                ps = PSH.tile([128, TOK], F32, tag="ps_h")
                for kp in range(4):
                    te.matmul(ps[:],
                              wff1_s[:, c * 8 + 2 * kp:c * 8 + 2 * kp + 2, :],
                              x1b8[:, 2 * kp:2 * kp + 2, :],
                              start=(kp == 0), stop=(kp == 3),
                              perf_mode=PM.DoubleRow)
                hs = hb[:, TOK * c:TOK * (c + 1)]
                v.tensor_scalar(hs, ps[:], 1.0 / (SW1 * SX1),
                                blc("b_ff1", c), Alu.mult, Alu.add)
                s.activation(hsqp[:], hs, Act.Square)
                te.matmul(sh, onesb[:], hs, start=(c == 0), stop=(c == 31))
                te.matmul(sh2, onesb[:], hsqp[:], start=(c == 0), stop=(c == 31))
            # ep path
            wep2_s = T3B.tile([128, 2], BF, tag="wep2_s")
            nc.sync.dma_start(out=wep2_s[:], in_=tin["wep2"].ap())
            epb = T3B.tile([128, 2 * TOK], BF, tag="epb")
            epsq = T3B.tile([128, TOK], BF, tag="epsq")
            for of in range(2):
                ps = PSH.tile([128, TOK], F32, tag="ps_h")
                for kc in range(32):
                    te.matmul(
                        ps[:],
                        wep1_s[:, (of * 32 + kc) * 128:(of * 32 + kc + 1) * 128],
                        hb[:, TOK * kc:TOK * (kc + 1)],
                        start=(kc == 0), stop=(kc == 31))
                s.activation(epb[:, TOK * of:TOK * (of + 1)], ps[:],
                             Act.Identity, bias=blc("b_ep1", of))
                v.tensor_tensor(epsq[:], epb[:, TOK * of:TOK * (of + 1)],
                                epb[:, TOK * of:TOK * (of + 1)], Alu.mult)
                te.matmul(se1, onesb[:], epb[:, TOK * of:TOK * (of + 1)],
                          start=(of == 0), stop=(of == 1))
                te.matmul(se2, onesb[:], epsq[:],
                          start=(of == 0), stop=(of == 1))
            _ln_rows(nc, rs(3), rs(4), rs(5), se1, se2, D16, cst[0:1, 0:1])
            mue_b = T3B.tile([128, TOK], F32, tag="mue_b")
            see_b = T3B.tile([128, TOK], F32, tag="see_b")
            g.partition_broadcast(mue_b[:], rs(3))
            g.partition_broadcast(see_b[:], rs(4))
            relub = T3B.tile([128, 2 * TOK], BF, tag="relub")
            tm3 = T3B.tile([128, TOK], F32, tag="tm3")
            for of in range(2):
                v.tensor_tensor(tm3[:], epb[:, TOK * of:TOK * (of + 1)],
                                mue_b[:], Alu.subtract)
                v.tensor_tensor(tm3[:], tm3[:], see_b[:], Alu.mult)
                v.tensor_scalar(tm3[:], tm3[:], blc("eplw", of),
                                blc("eplb", of), Alu.mult, Alu.add)
                v.tensor_scalar(relub[:, TOK * of:TOK * (of + 1)], tm3[:],
                                0.0, None, Alu.max)
            for of in range(2):
                te.matmul(pse2, wep2_s[:, of:of + 1],
                          relub[:, TOK * of:TOK * (of + 1)],
                          start=(of == 0), stop=(of == 1))
            erow = rs(6)
            s.activation(erow, pse2, Act.Sigmoid, bias=cst[0:1, 2:3])
            # emrep = SA * (1 + 0.1 e)
            v.tensor_scalar(erow, erow, 0.1 * SA, SA, Alu.mult, Alu.add)

            # spline per-token rows: mu_h (7), S (8)
            _spline_rows(nc, rs, sh, sh2, cst[0:1, 0:1])

            v.tensor_scalar(rs(0), rs(6), c0eff, None, Alu.mult)
            v.tensor_scalar(rs(1), rs(0), -64.0, SQ, Alu.mult, Alu.add)
            v.tensor_scalar(rs(2), rs(6), 64.0 * sgn2, None, Alu.mult)
            muh_b = T3B.tile([128, TOK], F32, tag="muh_b")
            Sh_b = T3B.tile([128, TOK], F32, tag="Sh_b")
            em_b = T3B.tile([128, TOK], F32, tag="em_b")
            thr_b = T3B.tile([128, TOK], F32, tag="thr_b")
            g.partition_broadcast(muh_b[:], rs(7))
            g.partition_broadcast(Sh_b[:], rs(8))
            g.partition_broadcast(em_b[:], rs(2))
            g.partition_broadcast(thr_b[:], rs(1))
            for (src, dst) in ((muh_b, murep), (Sh_b, Srep), (em_b, emqrep),
                               (thr_b, thrrep)):
                v.tensor_copy(dst[:], src[:].unsqueeze(1).to_broadcast((128, 8, TOK)))
        W6_cm.__exit__(None, None, None)
        # ============ Phase 7: spline + ff2 interleaved ============
        W7_cm = tc.tile_pool(name="w7_pool", bufs=1)
        W7 = W7_cm.__enter__()
        wff2_s = W7.tile([128, 256, 128], FP8, tag="wff2_s")
        for q in range(4):
            pq[q](out=wff2_s[:, 64 * q:64 * (q + 1), :],
                  in_=tin["wff2"].ap()[:, 8192 * q:8192 * (q + 1)])
        actt8 = W7.tile([128, 32, TOK], FP8, tag="actt8")
        with tc.tile_pool(name="spl", bufs=2) as SPL, \
             tc.tile_pool(name="ps_f2", bufs=1, space="PSUM") as PSF:
            accs = [PSF.tile([128, TOK], F32, tag=f"acc{of}", name=f"acc{of}")
                    for of in range(8)]
            us = [SPL.tile([128, 2048], BF, tag=f"u{gi}", name=f"u{gi}",
                           bufs=1) for gi in range(4)]
            p1s = [SPL.tile([128, 2048], BF, tag=f"p1{gi}", name=f"p1{gi}",
                            bufs=1) for gi in range(4)]
            for gi in range(4):
                hbs = hb[:, 2048 * gi:2048 * (gi + 1)]
                eng = g if gi % 2 == 0 else v
                eng.tensor_tensor(us[gi][:], hbs, murep[:], Alu.subtract)
                eng.tensor_tensor(us[gi][:], us[gi][:], Srep[:], Alu.mult)
            for gi in range(4):
                s.activation(p1s[gi][:], us[gi][:], Act.Square,
                             bias=cst[:, 3:4], scale=sgn2 * sq_s)
            inner_op = Alu.add if sgn2 * sgn3 > 0 else Alu.subtract
            for gi in range(4):
                acc = SPL.tile([128, 2048], BF, tag="acc", bufs=2)
                s.activation(acc[:], us[gi][:], Act.Abs, scale=abs(c3))
                v.tensor_tensor(acc[:], p1s[gi][:], acc[:], inner_op)
                v.tensor_tensor(acc[:], acc[:], emqrep[:], Alu.mult)
                v.tensor_tensor(
                    actt8[:, 8 * gi:8 * (gi + 1), :],
                    acc[:], thrrep[:], Alu.min)
                for of in range(8):
                    for kp in range(4):
                        kk = 8 * gi + 2 * kp
                        te.matmul(accs[of][:],
                                  wff2_s[:, of * 32 + kk:of * 32 + kk + 2, :],
                                  actt8[:, kk:kk + 2, :],
                                  start=(gi == 0 and kp == 0),
                                  stop=(gi == 3 and kp == 3),
                                  perf_mode=PM.DoubleRow)
            em0c = SPL.tile([128, TOK], F32, tag="em0c", bufs=1)
            g.partition_broadcast(em0c[:], rs(0))
            for of in range(8):
                v.tensor_scalar(x1f[:, TOK * of:TOK * (of + 1)],
                                x1f[:, TOK * of:TOK * (of + 1)],
                                blc("b_ff2", of), None, Alu.add)
                v.scalar_tensor_tensor(x1f[:, TOK * of:TOK * (of + 1)],
                                       accs[of][:], 1.0 / (SQ * SW2),
                                       x1f[:, TOK * of:TOK * (of + 1)],
                                       Alu.mult, Alu.add)
                v.scalar_tensor_tensor(x1f[:, TOK * of:TOK * (of + 1)],
                                       em0c[:], blc("rsw2", of),
                                       x1f[:, TOK * of:TOK * (of + 1)],
                                       Alu.mult, Alu.add)
        W7_cm.__exit__(None, None, None)
        with tc.tile_pool(name="ps_r4", bufs=1, space="PSUM") as PSR4, \
             tc.tile_pool(name="tmp4", bufs=2) as TMP4:
            _ln_full(nc, tc, TMP4, PSR4, rows, x1f, x1f, None, ones32,
                     blob, "n2w", "n2b", None, None, cst[0:1, 0:1])
        TMP3_cm.__exit__(None, None, None)
        nc.sync.dma_start(out=t_out.ap(), in_=x1f[:])


def _spline_rows(nc, rs, sh, sh2, epsap):
    """rs(7) = mu_h, rs(8) = S = 1/(sqrt(var+eps)*(norm+1)),
    norm = sqrt(FF*var/(var+eps) + eps)."""
    v, s = nc.vector, nc.scalar
    mu = rs(7)
    S_ = rs(8)
    var = rs(13)
    t1 = rs(14)
    t2 = rs(15)
    v.tensor_scalar(mu, sh, 1.0 / FF, None, Alu.mult)
    v.tensor_tensor(var, mu, mu, Alu.mult)
    v.tensor_scalar(t1, sh2, 1.0 / FF, None, Alu.mult)
    v.tensor_tensor(var, t1, var, Alu.subtract)
    s.activation(t1, var, Act.Ln, bias=epsap)
    s.activation(t1, t1, Act.Exp, scale=0.5)          # sqrt(var+eps)
    v.tensor_scalar(t2, var, EPS, None, Alu.add)
    v.reciprocal(t2, t2)
    v.tensor_tensor(t2, t2, var, Alu.mult)
    v.tensor_scalar(t2, t2, float(FF), None, Alu.mult)
    s.activation(t2, t2, Act.Ln, bias=epsap)
    s.activation(t2, t2, Act.Exp, scale=0.5)          # norm
    v.tensor_scalar(t2, t2, 1.0, None, Alu.add)
    v.tensor_tensor(t2, t2, t1, Alu.mult)
    v.reciprocal(S_, t2)


def _ln_full(nc, tc, TMP, PSR, rows, src, dstf, dst8, ones32, blob,
             wnm, bnm, w16nm, b16nm, epsap):
    co = lambda nm, k: blob[:, _CO[nm] + k:_CO[nm] + k + 1]
    v, s, g, te = nc.vector, nc.scalar, nc.gpsimd, nc.tensor
    T = TOK
    rs = lambda k: rows[0:1, k * T:(k + 1) * T]
    t_sx = PSR.tile([1, T], F32, tag="lnsxp")
    t_sx2 = PSR.tile([1, T], F32, tag="lnsx2p")
    sx, sx2 = t_sx[:], t_sx2[:]
    onesbl = TMP.tile([128, 1], mybir.dt.bfloat16, tag="lnonesb")
    v.memset(onesbl[:], 1.0)
    srcb = TMP.tile([128, 8 * T], mybir.dt.bfloat16, tag="lnsrcb")
    for kc in range(8):
        (v if kc % 2 == 0 else g).tensor_copy(
            srcb[:, T * kc:T * (kc + 1)], src[:, T * kc:T * (kc + 1)])
    for kc in range(8):
        te.matmul(sx, onesbl[:], srcb[:, T * kc:T * (kc + 1)],
                  start=(kc == 0), stop=(kc == 7))
    xsq = TMP.tile([128, T], mybir.dt.bfloat16, tag="lnxsq")
    for kc in range(8):
        v.tensor_tensor(xsq[:], srcb[:, T * kc:T * (kc + 1)],
                        srcb[:, T * kc:T * (kc + 1)], Alu.mult)
        te.matmul(sx2, onesbl[:], xsq[:], start=(kc == 0), stop=(kc == 7))
    _ln_rows(nc, rs(9), rs(10), rs(11), sx, sx2, D, epsap)
    mu_b = TMP.tile([128, T], F32, tag="lnmu_b")
    s_b = TMP.tile([128, T], F32, tag="lns_b")
    g.partition_broadcast(mu_b[:], rs(9))
    g.partition_broadcast(s_b[:], rs(10))
    tm = TMP.tile([128, T], F32, tag="lntm")
    for kc in range(8):
        v.tensor_tensor(tm[:], src[:, T * kc:T * (kc + 1)], mu_b[:],
                        Alu.subtract)
        v.tensor_tensor(tm[:], tm[:], s_b[:], Alu.mult)
        v.tensor_scalar(dstf[:, T * kc:T * (kc + 1)], tm[:],
                        co(wnm, kc), co(bnm, kc),
                        Alu.mult, Alu.add)
        if dst8 is not None:
            s.activation(dst8[:, kc:kc + 1, :], tm[:], Act.Identity,
                         bias=co(b16nm, kc), scale=co(w16nm, kc))


# ----------------------------------------------------------------------------
# Entry point
# ----------------------------------------------------------------------------

def kernel(**inputs):
    in_maps, sc = _prepare_inputs(inputs)
    key = hashlib.sha256(
        repr((sc["coeffs"], sc["ent_b"], sc["ep2_b"])).encode()
    ).hexdigest()
    if key not in _prog_cache:
        _prog_cache[key] = _build_program(sc)
    nc = _prog_cache[key]
    res = bass_utils.run_bass_kernel_spmd(nc, in_maps,
                                          core_ids=list(range(NCORES)))
    out = np.empty((1, S, D), np.float32)
    for c in range(NCORES):
        oc = np.asarray(res.results[c]["out"], np.float32)   # [128, 8*TOK]
        ot = oc.reshape(128, 8, TOK).transpose(1, 0, 2).reshape(D, TOK)
        out[0, c * TOK:(c + 1) * TOK, :] = ot.T
    return out


# revision 24
# speedup vs baseline: 1.3234x; 1.0564x over previous
"""Trainium2 8-core kernel for nn_EnhancedTransformerBlock (v2).

SPMD: identical program on all 8 cores.
  - Full x replicated to every core (bf16) -> no AllGather. Each core
    computes LN stats for all 2048 tokens, normalizes x, then QKV for its
    2 heads (head-sharded attention over the full sequence).
  - ln_attn affine and softmax temperature folded into QKV weights host-side.
  - Entropy gate folded into the V GEMM as an extra output column.
  - Attention: unshifted exp, denominator via ones-column on V, causal
    triangle masks, single reciprocal for all 8 (head, q-group) denominators.
  - AllToAll of per-head attention outputs back to sequence sharding
    (core c owns tokens [256c, 256c+256) for the FFN part).
  - ff1/ff2 GEMMs in fp8(e4m3) DoubleRow mode (K=256 per instruction,
    2x rate); activations scaled x16/x64 to sit in fp8's normal range.
  - Spline activation approximated by a 4-term kink-basis LSQ fit of the
    fixed 1-D function g(u) (computed host-side from runtime knots/spl_w);
    evaluated in ~9 elementwise ops per 2048-col group, alternating
    Vector/GpSimd engines; ff2 partial GEMMs interleaved with spline groups.
"""

import hashlib
import numpy as np

from concourse import bacc, tile, mybir
from concourse import bass_utils

dt = mybir.dt
BF = dt.bfloat16
F32 = dt.float32
FP8 = dt.float8e4
NPBF = dt.np(BF)
NPF8 = dt.np(FP8)
Alu = mybir.AluOpType
Act = mybir.ActivationFunctionType
PM = mybir.MatmulPerfMode

NCORES = 8
S = 2048
D = 1024
H = 16
HD = 64
FF = 4096
D16 = 256
TOK = S // NCORES            # 256 tokens per core
HPC = H // NCORES            # 2 heads per core
EPS = 1e-6
UDOM = 0.12                  # spline fit domain |u| <= UDOM (|u| < 0.09 true)

SX1 = 16.0                   # x1 fp8 scale
SW1 = 64.0                   # ff1_w fp8 scale
SA = 64.0                    # act fp8 scale
SW2 = 64.0                   # ff2_w fp8 scale
SQ = 4096.0                  # spline-delta fp8 scale

_prog_cache = {}


# ----------------------------------------------------------------------------
# Host-side: spline fit
# ----------------------------------------------------------------------------

def _g_exact(u, knots, spl_w):
    d = np.abs(u[:, None] - knots[None, :])
    d = d / (d.max(-1, keepdims=True) + EPS)
    a = -5.0 * d
    a = a - a.max(-1, keepdims=True)
    e = np.exp(a)
    p = e / e.sum(-1, keepdims=True)
    return (p * spl_w).sum(-1)


def _fit_spline(knots, spl_w):
    """LSQ fit of g(u) on [-UDOM, UDOM]; basis [1, u, u^2, |u|]."""
    k = np.asarray(knots, np.float64)
    w = np.asarray(spl_w, np.float64)
    u = np.linspace(-UDOM, UDOM, 20001)
    B = np.stack([np.ones_like(u), u, u * u, np.abs(u)], -1)
    y = _g_exact(u, k, w)
    c, *_ = np.linalg.lstsq(B, y, rcond=None)
    err = float(np.abs(B @ c - y).max())
    return [float(v) for v in c], err


# ----------------------------------------------------------------------------
# Host-side: weight packing
# ----------------------------------------------------------------------------

def _pack_lhsT(w_t, n_of, n_kc):
    """w_t: [K_total, M_total] ([in, out]) -> [128, n_of*n_kc*128], tile
    (of, kc) at cols [(of*n_kc+kc)*128 ...] = w_t[128kc:.., 128of:..]."""
    K_total, M_total = w_t.shape
    assert K_total == n_kc * 128 and M_total == n_of * 128
    out = np.empty((128, n_of * n_kc * 128), np.float32)
    for of in range(n_of):
        for kc in range(n_kc):
            out[:, (of * n_kc + kc) * 128:(of * n_kc + kc + 1) * 128] = \
                w_t[kc * 128:(kc + 1) * 128, of * 128:(of + 1) * 128]
    return np.ascontiguousarray(out)


def _col_pack(vec, n_chunks):
    return np.ascontiguousarray(
        np.asarray(vec, np.float32).reshape(n_chunks, 128).T)


def _make_tri_masks():
    out = np.zeros((128, 4 * 512), np.float32)
    for j in range(4):
        kk = np.arange(128)[:, None] + 128 * j
        q = np.arange(512)[None, :]
        out[:, 512 * j:512 * (j + 1)] = (kk <= q).astype(np.float32)
    return out


# const blob layout (f32 [128, CW]); col offsets
_CO = {}
_cw = 0
def _co(name, w):
    global _cw
    _CO[name] = _cw
    _cw += w
_co("b_qk", 2)
_co("b_out", 8)
_co("b_ff1", 32)
_co("b_ff2", 8)
_co("b_ep1", 2)
_co("n1w", 8)
_co("n1b", 8)
_co("n1w16", 8)
_co("n1b16", 8)
_co("n2w", 8)
_co("n2b", 8)
_co("eplw", 2)
_co("eplb", 2)
_co("rsw2", 8)
_co("bv_row", 137)           # row 0 only
CW = _cw


def _prepare_inputs(inputs):
    f = lambda k: np.asarray(inputs[k], np.float32)
    x = f("x").reshape(S, D)
    qkv_w, qkv_b = f("qkv_w"), f("qkv_b")
    out_w, out_b = f("out_w") * 0.1, f("out_b") * 0.1
    ff1_w, ff1_b = f("ff1_w"), f("ff1_b")
    ff2_w, ff2_b = f("ff2_w"), f("ff2_b")
    ep1_w, ep1_b = f("ep1_w"), f("ep1_b")
    ep2_w, ep2_b = f("ep2_w"), f("ep2_b")
    ent_w, ent_b = f("ent_w"), f("ent_b")
    lnw, lnb = f("ln_attn_w"), f("ln_attn_b")
    n1w, n1b = f("norm1_w"), f("norm1_b")

    temp = (1.0 / np.sqrt(np.float32(HD))) / 0.1   # 1.25
    # fold ln_attn affine into qkv/ent weights: W'(xn) + b' == W(xl) + b
    wq = qkv_w[0:D] * temp * lnw[None, :]
    wk = qkv_w[D:2 * D] * lnw[None, :]
    wv = qkv_w[2 * D:3 * D] * lnw[None, :]
    bq = qkv_b[0:D] * temp + wq @ lnb
    bk = qkv_b[D:2 * D] + wk @ lnb
    bv = qkv_b[2 * D:3 * D] + wv @ lnb
    went = ent_w.reshape(D) * lnw
    bent = float(ent_b.reshape(-1)[0] + went @ lnb)

    coeffs, fit_err = _fit_spline(f("knots"), f("spl_w"))

    xT = np.ascontiguousarray(x.T)                           # [D, S]
    xfull = np.ascontiguousarray(
        xT.reshape(8, 128, S).transpose(1, 0, 2).reshape(128, 8 * S)
    ).astype(NPBF)

    shared = {
        "xfull": xfull,
        "tri": _make_tri_masks().astype(NPBF),
        "wout": _pack_lhsT(out_w.T, 8, 8).astype(NPBF),
        "wff1": np.ascontiguousarray(
            _pack_lhsT(ff1_w.T * SW1, 32, 8)).astype(NPF8),
        "wff2": np.ascontiguousarray(
            _pack_lhsT(ff2_w.T * SW2, 8, 32)).astype(NPF8),
        "wep1": _pack_lhsT(ep1_w.T, 2, 32).astype(NPBF),
        "wep2": np.ascontiguousarray(
            ep2_w.reshape(2, 128).T).astype(NPBF),          # [128, 2]
    }

    scalars = {
        "ent_b": bent,
        "ep2_b": float(ep2_b.reshape(-1)[0]),
        "coeffs": coeffs,
        "fit_err": fit_err,
    }

    in_maps = []
    for c in range(NCORES):
        m = dict(shared)
        xc = x[c * TOK:(c + 1) * TOK]                        # [256, D]
        xTc = np.ascontiguousarray(xc.T)                     # [D, 256]
        m["xT"] = np.ascontiguousarray(
            xTc.reshape(8, 128, TOK).transpose(1, 0, 2).reshape(128, 8 * TOK))
        h0 = c * HPC
        wq_c = wq[h0 * HD:(h0 + HPC) * HD]                   # [128, D]
        wk_c = wk[h0 * HD:(h0 + HPC) * HD]
        wqk_t = np.concatenate([wq_c, wk_c], 0).T            # [D, 256]
        m["wqk"] = _pack_lhsT(wqk_t, 2, 8).astype(NPBF)
        # V weights [D, 137]: v_h0(64) pad(4) v_h1(64) pad(4) went(1)
        wv_c = wv[h0 * HD:(h0 + HPC) * HD].T                 # [D, 128]
        wva = np.zeros((D, 137), np.float32)
        bva = np.zeros((1, 137), np.float32)
        for lh in range(HPC):
            wva[:, 68 * lh:68 * lh + 64] = wv_c[:, 64 * lh:64 * lh + 64]
            bva[0, 68 * lh:68 * lh + 64] = \
                bv[(h0 + lh) * HD:(h0 + lh + 1) * HD]
        wva[:, 136] = went
        m["wv"] = np.ascontiguousarray(
            wva.reshape(8, 128, 137).transpose(1, 0, 2).reshape(128, 8 * 137)
        ).astype(NPBF)
        # const blob
        blob = np.zeros((128, CW), np.float32)
        def put(name, arr):
            a = np.asarray(arr, np.float32)
            blob[:, _CO[name]:_CO[name] + a.shape[1]] = a
        put("b_qk", np.stack([bq[h0 * HD:(h0 + HPC) * HD],
                              bk[h0 * HD:(h0 + HPC) * HD]], -1))
        put("b_out", _col_pack(out_b, 8))
        put("b_ff1", _col_pack(ff1_b, 32))
        put("b_ff2", _col_pack(ff2_b, 8))
        put("b_ep1", _col_pack(ep1_b, 2))
        put("n1w", _col_pack(n1w, 8))
        put("n1b", _col_pack(n1b, 8))
        put("n1w16", _col_pack(n1w * SX1, 8))
        put("n1b16", _col_pack(n1b * SX1, 8))
        put("n2w", _col_pack(f("norm2_w"), 8))
        put("n2b", _col_pack(f("norm2_b"), 8))
        put("eplw", _col_pack(f("ep_ln_w"), 2))
        put("eplb", _col_pack(f("ep_ln_b"), 2))
        put("rsw2", _col_pack(ff2_w.sum(1) / SA, 8))
        blob[0, _CO["bv_row"]:_CO["bv_row"] + 137] = bva[0]
        m["blob"] = np.ascontiguousarray(blob)
        in_maps.append(m)

    return in_maps, scalars


# ----------------------------------------------------------------------------
# Device program
# ----------------------------------------------------------------------------

def _build_program(sc):
    nc = bacc.Bacc("TRN2", target_bir_lowering=False, debug=False,
                   num_devices=NCORES)

    def din(name, shape, dtype):
        return nc.dram_tensor(name, list(shape), dtype, kind="ExternalInput")

    tin = {
        "xfull": din("xfull", (128, 8 * S), BF),
        "xT": din("xT", (128, 8 * TOK), F32),
        "wqk": din("wqk", (128, 2048), BF),
        "wv": din("wv", (128, 8 * 137), BF),
        "wout": din("wout", (128, 8192), BF),
        "wff1": din("wff1", (128, 32768), FP8),
        "wff2": din("wff2", (128, 32768), FP8),
        "wep1": din("wep1", (128, 8192), BF),
        "wep2": din("wep2", (128, 2), BF),
        "tri": din("tri", (128, 2048), BF),
        "blob": din("blob", (128, CW), F32),
    }
    t_out = nc.dram_tensor("out", [128, 8 * TOK], F32, kind="ExternalOutput")
    a2a_in = nc.dram_tensor("a2a_in", [1024, TOK], BF, kind="Internal")
    a2a_out = nc.dram_tensor("a2a_out", [1024, TOK], BF, kind="Internal")

    with tile.TileContext(nc) as tc:
        _emit(nc, tc, tin, t_out, a2a_in, a2a_out, sc)
    nc.compile()
    return nc


def _ln_rows(nc, mu, st, tmp, sx, sx2, n, epsap):
    """mu = sx/n; st = 1/sqrt(var+eps) with var = sx2/n - mu^2."""
    v, s = nc.vector, nc.scalar
    v.tensor_scalar(mu, sx, 1.0 / n, None, Alu.mult)
    v.tensor_tensor(st, mu, mu, Alu.mult)
    v.tensor_scalar(tmp, sx2, 1.0 / n, None, Alu.mult)
    v.tensor_tensor(st, tmp, st, Alu.subtract)
    v.tensor_scalar(st, st, EPS, None, Alu.add)
    v.reciprocal(st, st)
    s.activation(st, st, Act.Sqrt)


def _emit(nc, tc, tin, t_out, a2a_in, a2a_out, sc):
    v = nc.vector
    s = nc.scalar
    g = nc.gpsimd
    te = nc.tensor
    c0, c1, c2, c3 = sc["coeffs"]
    # q(u) = c1 u + c2 u^2 + c3|u| == sgn2*Square(sq_s*u + sq_b)
    #        + sgn3*Abs(|c3| u) - sgn2*sq_b^2
    sgn2 = 1.0 if c2 >= 0 else -1.0
    sgn3 = 1.0 if c3 >= 0 else -1.0
    sq_s = float(np.sqrt(max(abs(c2), 1e-6)))
    sq_b = c1 / (2.0 * sq_s * sgn2)
    c0eff = c0 - sgn2 * sq_b * sq_b
    RG = [list(range(NCORES))]

    with tc.tile_pool(name="persist", bufs=1) as P, \
         tc.tile_pool(name="consts", bufs=1) as C, \
         tc.tile_pool(name="rows", bufs=1) as R:

        # ---- constants + input DMAs, spread across queues ----
        blob = C.tile([128, CW], F32, tag="blob")
        nc.sync.dma_start(out=blob[:], in_=tin["blob"].ap())
        blc = lambda nm, k: blob[:, _CO[nm] + k:_CO[nm] + k + 1]

        tri = C.tile([128, 2048], BF, tag="tri")
        g.dma_start(out=tri[:], in_=tin["tri"].ap())
        xt = P.tile([128, 8 * TOK], F32, tag="xt")
        g.dma_start(out=xt[:], in_=tin["xT"].ap())

        XN_cm = tc.tile_pool(name="xn_pool", bufs=1)
        XN = XN_cm.__enter__()
        xn = XN.tile([128, 8 * S], BF, tag="xn")
        XF_cm = tc.tile_pool(name="xf_pool", bufs=1)
        XF = XF_cm.__enter__()
        xf = XF.tile([128, 8 * S], BF, tag="xf")
        qdma = [nc.sync.dma_start, nc.scalar.dma_start,
                nc.gpsimd.dma_start, nc.sync.dma_start]
        for q in range(4):
            qdma[q](out=xf[:, 4096 * q:4096 * (q + 1)],
                    in_=tin["xfull"].ap()[:, 4096 * q:4096 * (q + 1)])

        onesb = C.tile([128, 1], BF, tag="onesb")
        ones32 = C.tile([128, 1], F32, tag="ones32")
        onesr = C.tile([1, 64], BF, tag="onesr")
        v.memset(onesb[:], 1.0)
        v.memset(ones32[:], 1.0)
        v.memset(onesr[:], 1.0)
        cst = C.tile([128, 4], F32, tag="cst")
        v.memset(cst[:, 0:1], EPS)
        v.memset(cst[:, 1:2], sc["ent_b"])
        v.memset(cst[:, 2:3], sc["ep2_b"])
        v.memset(cst[:, 3:4], sgn2 * sq_b)
        bvb = C.tile([128, 137], F32, tag="bvb")
        g.partition_broadcast(bvb[:], blob[0:1, _CO["bv_row"]:_CO["bv_row"] + 137])

        # persistent activations
        qkT = P.tile([128, 4096], BF, tag="qkT")
        vaug = P.tile([128, 16 * 137], BF, tag="vaug")
        aosc = P.tile([128, 2048], BF, tag="aosc")
        aofull = P.tile([128, 8 * TOK], BF, tag="aofull")
        x1f = P.tile([128, 8 * TOK], F32, tag="x1f")
        x1b8 = P.tile([128, 8, TOK], FP8, tag="x1b8")
        rows = R.tile([1, 16 * TOK], F32, tag="rows")
        rs = lambda k: rows[0:1, k * TOK:(k + 1) * TOK]
        denpA = R.tile([128, 512], F32, tag="denpA")
        denpB = R.tile([128, 512], F32, tag="denpB")

        # ============ Phase 1: full-seq LN stats + normalize ============
        with tc.tile_pool(name="ps_r1", bufs=4, space="PSUM") as PSR, \
             tc.tile_pool(name="tmp1", bufs=1) as TMP:
            mu_r = TMP.tile([1, S], F32, tag="mu_r")
            st_r = TMP.tile([1, S], F32, tag="st_r")
            tm_r = TMP.tile([1, S], F32, tag="tm_r")
            for w in range(4):
                sx = PSR.tile([1, 512], F32, tag="sx1p")
                sx2 = PSR.tile([1, 512], F32, tag="sx2p")
                sx, sx2 = sx[:], sx2[:]
                for kc in range(8):
                    te.matmul(sx, onesb[:],
                              xf[:, 2048 * kc + 512 * w:2048 * kc + 512 * (w + 1)],
                              start=(kc == 0), stop=(kc == 7))
                for kc in range(8):
                    xsq = TMP.tile([128, 512], BF, tag="xsq", bufs=4)
                    eng = v if kc % 2 == 0 else g
                    eng.tensor_tensor(
                        xsq[:], xf[:, 2048 * kc + 512 * w:2048 * kc + 512 * (w + 1)],
                        xf[:, 2048 * kc + 512 * w:2048 * kc + 512 * (w + 1)],
                        Alu.mult)
                    te.matmul(sx2, onesb[:], xsq[:],
                              start=(kc == 0), stop=(kc == 7))
                v.tensor_scalar(mu_r[0:1, 512 * w:512 * (w + 1)], sx,
                                1.0 / D, None, Alu.mult)
                v.tensor_scalar(st_r[0:1, 512 * w:512 * (w + 1)], sx2,
                                1.0 / D, None, Alu.mult)
            v.tensor_tensor(tm_r[:], mu_r[:], mu_r[:], Alu.mult)
            v.tensor_tensor(st_r[:], st_r[:], tm_r[:], Alu.subtract)
            s.activation(st_r[:], st_r[:], Act.Ln, bias=cst[0:1, 0:1])
            s.activation(st_r[:], st_r[:], Act.Exp, scale=-0.5)
            v.tensor_tensor(tm_r[:], mu_r[:], st_r[:], Alu.mult)
            # bf16 reps
            str_b = TMP.tile([1, S], BF, tag="str_b")
            mst_b = TMP.tile([1, S], BF, tag="mst_b")
            v.tensor_copy(str_b[:], st_r[:])
            v.tensor_copy(mst_b[:], tm_r[:])
            strep = TMP.tile([128, S], BF, tag="strep")
            mstrep = TMP.tile([128, S], BF, tag="mstrep")
            g.partition_broadcast(strep[:], str_b[:])
            g.partition_broadcast(mstrep[:], mst_b[:])
            for kc in range(8):
                tm = TMP.tile([128, S], BF, tag="nrm", bufs=2)
                eng = g if kc in (2, 5) else v
                eng.tensor_tensor(tm[:], xf[:, 2048 * kc:2048 * (kc + 1)],
                                  strep[:], Alu.mult)
                eng.tensor_tensor(xn[:, 2048 * kc:2048 * (kc + 1)],
                                  tm[:], mstrep[:], Alu.subtract)
        XF_cm.__exit__(None, None, None)

        # ============ Phase 2: QKV + ent ============
        with tc.tile_pool(name="wq_pool", bufs=1) as WQ, \
             tc.tile_pool(name="ps_qk", bufs=2, space="PSUM") as PSQ, \
             tc.tile_pool(name="ps_ev", bufs=3, space="PSUM") as PSV, \
             tc.tile_pool(name="estmp", bufs=4) as EST:
            wv_s = WQ.tile([128, 8 * 137], BF, tag="wv_s")
            nc.scalar.dma_start(out=wv_s[:], in_=tin["wv"].ap())
            wqk_s = WQ.tile([128, 2048], BF, tag="wqk_s")
            nc.sync.dma_start(out=wqk_s[:], in_=tin["wqk"].ap())

            for tch in range(16):
                psv = PSV.tile([128, 137], F32, tag="psv", bufs=2)
                for kc in range(8):
                    te.matmul(
                        psv[:],
                        xn[:, 2048 * kc + 128 * tch:2048 * kc + 128 * (tch + 1)],
                        wv_s[:, 137 * kc:137 * (kc + 1)],
                        start=(kc == 0), stop=(kc == 7))
                esc = EST.tile([128, 1], F32, tag="esc")
                s.activation(esc[:], psv[:, 136:137], Act.Sigmoid,
                             bias=cst[:, 1:2])
                v.tensor_scalar(esc[:], esc[:], 0.1, None, Alu.max)
                vt = vaug[:, 137 * tch:137 * tch + 136]
                v.tensor_tensor(vt, psv[:, 0:136], bvb[:, 0:136], Alu.add)
                v.tensor_scalar(vt, vt, esc[:], None, Alu.mult)
                for lh in range(HPC):
                    v.memset(vaug[:, 137 * tch + 68 * lh + 64:
                                  137 * tch + 68 * lh + 65], 1.0)

            for of in range(2):
                for w in range(4):
                    ps = PSQ.tile([128, 512], F32, tag="psqk")
                    for kc in range(8):
                        te.matmul(
                            ps[:],
                            wqk_s[:, (of * 8 + kc) * 128:(of * 8 + kc + 1) * 128],
                            xn[:, 2048 * kc + 512 * w:2048 * kc + 512 * (w + 1)],
                            start=(kc == 0), stop=(kc == 7))
                    v.tensor_scalar(
                        qkT[:, 2048 * of + 512 * w:2048 * of + 512 * (w + 1)],
                        ps[:], blc("b_qk", of), None, Alu.add)
        XN_cm.__exit__(None, None, None)

        # ---- prefetch big weights (land during attention) ----
        TMP3_cm = tc.tile_pool(name="tmp3", bufs=1)
        TMP3 = TMP3_cm.__enter__()
        hb = TMP3.tile([128, 8192], BF, tag="hb")
        murep = TMP3.tile([128, 2048], BF, tag="murep")
        Srep = TMP3.tile([128, 2048], BF, tag="Srep")
        emqrep = TMP3.tile([128, 2048], BF, tag="emqrep")
        thrrep = TMP3.tile([128, 2048], BF, tag="thrrep")
        W6_cm = tc.tile_pool(name="w6_pool", bufs=1)
        W6 = W6_cm.__enter__()
        WO_cm = tc.tile_pool(name="wo_pool", bufs=1)
        WO = WO_cm.__enter__()
        wout_s = WO.tile([128, 8192], BF, tag="wout_s")
        wep1_s = W6.tile([128, 8192], BF, tag="wep1_s")
        wff1_s = W6.tile([128, 256, 128], FP8, tag="wff1_s")
        pq = [nc.sync.dma_start, nc.scalar.dma_start, nc.gpsimd.dma_start,
              nc.sync.dma_start]
        for q in range(4):
            pq[q](out=wout_s[:, 2048 * q:2048 * (q + 1)],
                  in_=tin["wout"].ap()[:, 2048 * q:2048 * (q + 1)])
        for q in range(4):
            pq[q](out=wff1_s[:, 64 * q:64 * (q + 1), :],
                  in_=tin["wff1"].ap()[:, 8192 * q:8192 * (q + 1)])
        for q in range(4):
            pq[q](out=wep1_s[:, 2048 * q:2048 * (q + 1)],
                  in_=tin["wep1"].ap()[:, 2048 * q:2048 * (q + 1)])

        # ============ Phase 3: attention ============
        att_stash = []
        with tc.tile_pool(name="ps_sc", bufs=2, space="PSUM") as PSS, \
             tc.tile_pool(name="ps_ao", bufs=2, space="PSUM") as PSA, \
             tc.tile_pool(name="att_sb", bufs=3) as ASB, \
             tc.tile_pool(name="ao_sb", bufs=8) as AOSB:
            for lh in range(HPC):
                hq = qkT[64 * lh:64 * (lh + 1), 0:2048]
                hk = qkT[64 * lh:64 * (lh + 1), 2048:4096]
                for G in range(4):
                    nkb = 4 * G + 4
                    ao = PSA.tile([65, 512], F32, tag="ao")
                    for pj in range(nkb // 2):
                        ps = PSS.tile([128, 1024], F32, tag="ps_sc")
                        ex = ASB.tile([128, 1024], BF, tag="ex")
                        for half in range(2):
                            kb = 2 * pj + half
                            te.matmul(ps[:, 512 * half:512 * (half + 1)],
                                      hk[:, 128 * kb:128 * (kb + 1)],
                                      hq[:, 512 * G:512 * (G + 1)],
                                      start=True, stop=True)
                        s.activation(ex[:], ps[:], Act.Exp)
                        for half in range(2):
                            kb = 2 * pj + half
                            j = kb - 4 * G
                            exh = ex[:, 512 * half:512 * (half + 1)]
                            if 0 <= j < 4:
                                v.tensor_tensor(
                                    exh, exh, tri[:, 512 * j:512 * (j + 1)],
                                    Alu.mult)
                            te.matmul(
                                ao[:],
                                vaug[:, 137 * kb + 68 * lh:
                                     137 * kb + 68 * lh + 65],
                                exh,
                                start=(kb == 0), stop=(kb == nkb - 1))
                    aos = AOSB.tile([65, 512], BF, tag="aos")
                    s.copy(aos[:], ao[0:65, :])
                    dent = denpA if lh == 0 else denpB
                    v.tensor_copy(dent[32 * G:32 * G + 1, :], aos[64:65, :])
                    att_stash.append((lh, G, aos))
            v.reciprocal(denpA[:], denpA[:])
            v.reciprocal(denpB[:], denpB[:])
            for lh, G, aos in att_stash:
                rrow = ASB.tile([1, 512], BF, tag="rrow")
                dent = denpA if lh == 0 else denpB
                v.tensor_copy(rrow[0:1, :], dent[32 * G:32 * G + 1, :])
                rbp = PSA.tile([64, 512], F32, tag="rbp")
                te.matmul(rbp[:], onesr[:], rrow[:], start=True, stop=True)
                v.tensor_tensor(
                    aosc[64 * lh:64 * (lh + 1), 512 * G:512 * (G + 1)],
                    aos[0:64, :], rbp[:], Alu.mult)

        # ============ Phase 4: AllToAll ============
        for r in range(NCORES):
            nc.sync.dma_start(out=a2a_in.ap()[128 * r:128 * (r + 1), :],
                              in_=aosc[:, TOK * r:TOK * (r + 1)])
        g.collective_compute("AllToAll", Alu.bypass, replica_groups=RG,
                             ins=[a2a_in.ap()], outs=[a2a_out.ap()])
        for r in range(NCORES):
            nc.sync.dma_start(out=aofull[:, TOK * r:TOK * (r + 1)],
                              in_=a2a_out.ap()[128 * r:128 * (r + 1), :])
        # ============ Phase 5: out proj + norm1 ============
        with tc.tile_pool(name="ps_out", bufs=3, space="PSUM") as PSO, \
             tc.tile_pool(name="ps_r2", bufs=1, space="PSUM") as PSR2, \
             tc.tile_pool(name="tmp2", bufs=2) as TMP2:
            for of in range(8):
                ps = PSO.tile([128, TOK], F32, tag="ps_out")
                for kc in range(8):
                    te.matmul(
                        ps[:],
                        wout_s[:, (of * 8 + kc) * 128:(of * 8 + kc + 1) * 128],
                        aofull[:, TOK * kc:TOK * (kc + 1)],
                        start=(kc == 0), stop=(kc == 7))
                v.scalar_tensor_tensor(xt[:, TOK * of:TOK * (of + 1)],
                                       ps[:], blc("b_out", of),
                                       xt[:, TOK * of:TOK * (of + 1)],
                                       Alu.add, Alu.add)
            _ln_full(nc, tc, TMP2, PSR2, rows, xt, x1f, x1b8, ones32,
                     blob, "n1w", "n1b", "n1w16", "n1b16", cst[0:1, 0:1])
        WO_cm.__exit__(None, None, None)

        # ============ Phase 6: ff1 (fp8) + ep path + spline rows ============
        with tc.tile_pool(name="ps_h", bufs=2, space="PSUM") as PSH, \
             tc.tile_pool(name="ps_r3", bufs=1, space="PSUM") as PSR3, \
             tc.tile_pool(name="tmp3b", bufs=1) as T3B:
            t_sh = PSR3.tile([1, TOK], F32, tag="shp")
            t_sh2 = PSR3.tile([1, TOK], F32, tag="sh2p")
            t_se1 = PSR3.tile([1, TOK], F32, tag="se1p")
            t_se2 = PSR3.tile([1, TOK], F32, tag="se2p")
            t_pse2 = PSR3.tile([1, TOK], F32, tag="pse2p")
            sh, sh2, se1, se2, pse2 = (t_sh[:], t_sh2[:], t_se1[:],
                                       t_se2[:], t_pse2[:])
            hsqp = T3B.tile([128, TOK], BF, tag="hsqp")
            for c in range(32):
                ps = PSH.tile([128, TOK], F32, tag="ps_h")
                for kp in range(4):
                    te.matmul(ps[:],
                              wff1_s[:, c * 8 + 2 * kp:c * 8 + 2 * kp + 2, :],
                              x1b8[:, 2 * kp:2 * kp + 2, :],
                              start=(kp == 0), stop=(kp == 3),
                              perf_mode=PM.DoubleRow)
                hs = hb[:, TOK * c:TOK * (c + 1)]
                s.activation(hs, ps[:], Act.Identity,
                             bias=blc("b_ff1", c), scale=1.0 / (SW1 * SX1))
                v.tensor_tensor(hsqp[:], hs, hs, Alu.mult)
                te.matmul(sh, onesb[:], hs, start=(c == 0), stop=(c == 31))
                te.matmul(sh2, onesb[:], hsqp[:], start=(c == 0), stop=(c == 31))
            # ep path
            wep2_s = T3B.tile([128, 2], BF, tag="wep2_s")
            nc.sync.dma_start(out=wep2_s[:], in_=tin["wep2"].ap())
            epb = T3B.tile([128, 2 * TOK], BF, tag="epb")
            epsq = T3B.tile([128, TOK], BF, tag="epsq")
            for of in range(2):
                ps = PSH.tile([128, TOK], F32, tag="ps_h")
                for kc in range(32):
                    te.matmul(
                        ps[:],
                        wep1_s[:, (of * 32 + kc) * 128:(of * 32 + kc + 1) * 128],
                        hb[:, TOK * kc:TOK * (kc + 1)],
                        start=(kc == 0), stop=(kc == 31))
                v.tensor_scalar(epb[:, TOK * of:TOK * (of + 1)], ps[:],
                                blc("b_ep1", of), None, Alu.add)
                v.tensor_tensor(epsq[:], epb[:, TOK * of:TOK * (of + 1)],
                                epb[:, TOK * of:TOK * (of + 1)], Alu.mult)
                te.matmul(se1, onesb[:], epb[:, TOK * of:TOK * (of + 1)],
                          start=(of == 0), stop=(of == 1))
                te.matmul(se2, onesb[:], epsq[:],
                          start=(of == 0), stop=(of == 1))
            _ln_rows(nc, rs(3), rs(4), rs(5), se1, se2, D16, cst[0:1, 0:1])
            # spline per-token rows: mu_h (7), S (8)  (before sigmoid: shares
            # the Sqrt table with the ep layernorm rows)
            _spline_rows(nc, rs, sh, sh2, cst[0:1, 0:1])
            mue_b = T3B.tile([128, TOK], F32, tag="mue_b")
            see_b = T3B.tile([128, TOK], F32, tag="see_b")
            g.partition_broadcast(mue_b[:], rs(3))
            g.partition_broadcast(see_b[:], rs(4))
            relub = T3B.tile([128, 2 * TOK], BF, tag="relub")
            tm3 = T3B.tile([128, TOK], F32, tag="tm3")
            for of in range(2):
                v.tensor_tensor(tm3[:], epb[:, TOK * of:TOK * (of + 1)],
                                mue_b[:], Alu.subtract)
                v.tensor_tensor(tm3[:], tm3[:], see_b[:], Alu.mult)
                v.tensor_scalar(tm3[:], tm3[:], blc("eplw", of),
                                blc("eplb", of), Alu.mult, Alu.add)
                v.tensor_scalar(relub[:, TOK * of:TOK * (of + 1)], tm3[:],
                                0.0, None, Alu.max)
            for of in range(2):
                te.matmul(pse2, wep2_s[:, of:of + 1],
                          relub[:, TOK * of:TOK * (of + 1)],
                          start=(of == 0), stop=(of == 1))
            erow = rs(6)
            s.activation(erow, pse2, Act.Sigmoid, bias=cst[0:1, 2:3])
            # emrep = SA * (1 + 0.1 e)
            v.tensor_scalar(erow, erow, 0.1 * SA, SA, Alu.mult, Alu.add)

            # spline per-token rows: mu_h (7), S (8)
            _spline_rows(nc, rs, sh, sh2, cst[0:1, 0:1])

            v.tensor_scalar(rs(0), rs(6), c0eff, None, Alu.mult)
            v.tensor_scalar(rs(1), rs(0), -64.0, SQ, Alu.mult, Alu.add)
            v.tensor_scalar(rs(2), rs(6), 64.0 * sgn2, None, Alu.mult)
            muh_b = T3B.tile([128, TOK], F32, tag="muh_b")
            Sh_b = T3B.tile([128, TOK], F32, tag="Sh_b")
            em_b = T3B.tile([128, TOK], F32, tag="em_b")
            thr_b = T3B.tile([128, TOK], F32, tag="thr_b")
            g.partition_broadcast(muh_b[:], rs(7))
            g.partition_broadcast(Sh_b[:], rs(8))
            g.partition_broadcast(em_b[:], rs(2))
            g.partition_broadcast(thr_b[:], rs(1))
            for (src, dst) in ((muh_b, murep), (Sh_b, Srep), (em_b, emqrep),
                               (thr_b, thrrep)):
                v.tensor_copy(dst[:], src[:].unsqueeze(1).to_broadcast((128, 8, TOK)))
        W6_cm.__exit__(None, None, None)
        # ============ Phase 7: spline + ff2 interleaved ============
        W7_cm = tc.tile_pool(name="w7_pool", bufs=1)
        W7 = W7_cm.__enter__()
        wff2_s = W7.tile([128, 256, 128], FP8, tag="wff2_s")
        for q in range(4):
            pq[q](out=wff2_s[:, 64 * q:64 * (q + 1), :],
                  in_=tin["wff2"].ap()[:, 8192 * q:8192 * (q + 1)])
        actt8 = W7.tile([128, 32, TOK], FP8, tag="actt8")
        with tc.tile_pool(name="spl", bufs=2) as SPL, \
             tc.tile_pool(name="ps_f2", bufs=1, space="PSUM") as PSF:
            accs = [PSF.tile([128, TOK], F32, tag=f"acc{of}", name=f"acc{of}")
                    for of in range(8)]
            us = [SPL.tile([128, 2048], BF, tag=f"u{gi}", name=f"u{gi}",
                           bufs=1) for gi in range(4)]
            p1s = [SPL.tile([128, 2048], BF, tag=f"p1{gi}", name=f"p1{gi}",
                            bufs=1) for gi in range(4)]
            for gi in range(4):
                hbs = hb[:, 2048 * gi:2048 * (gi + 1)]
                eng = g if gi % 2 == 0 else v
                eng.tensor_tensor(us[gi][:], hbs, murep[:], Alu.subtract)
                eng.tensor_tensor(us[gi][:], us[gi][:], Srep[:], Alu.mult)
            for gi in range(4):
                s.activation(p1s[gi][:], us[gi][:], Act.Square,
                             bias=cst[:, 3:4], scale=sgn2 * sq_s)
            inner_op = Alu.add if sgn2 * sgn3 > 0 else Alu.subtract
            for gi in range(4):
                acc = SPL.tile([128, 2048], BF, tag="acc", bufs=2)
                s.activation(acc[:], us[gi][:], Act.Abs, scale=abs(c3))
                v.tensor_tensor(acc[:], p1s[gi][:], acc[:], inner_op)
                v.tensor_tensor(acc[:], acc[:], emqrep[:], Alu.mult)
                v.tensor_tensor(
                    actt8[:, 8 * gi:8 * (gi + 1), :],
                    acc[:], thrrep[:], Alu.min)
                for of in range(8):
                    for kp in range(4):
                        kk = 8 * gi + 2 * kp
                        te.matmul(accs[of][:],
                                  wff2_s[:, of * 32 + kk:of * 32 + kk + 2, :],
                                  actt8[:, kk:kk + 2, :],
                                  start=(gi == 0 and kp == 0),
                                  stop=(gi == 3 and kp == 3),
                                  perf_mode=PM.DoubleRow)
            em0c = SPL.tile([128, TOK], F32, tag="em0c", bufs=1)
            g.partition_broadcast(em0c[:], rs(0))
            for of in range(8):
                v.tensor_scalar(x1f[:, TOK * of:TOK * (of + 1)],
                                x1f[:, TOK * of:TOK * (of + 1)],
                                blc("b_ff2", of), None, Alu.add)
                v.scalar_tensor_tensor(x1f[:, TOK * of:TOK * (of + 1)],
                                       accs[of][:], 1.0 / (SQ * SW2),
                                       x1f[:, TOK * of:TOK * (of + 1)],
                                       Alu.mult, Alu.add)
                v.scalar_tensor_tensor(x1f[:, TOK * of:TOK * (of + 1)],
                                       em0c[:], blc("rsw2", of),
                                       x1f[:, TOK * of:TOK * (of + 1)],
                                       Alu.mult, Alu.add)
        W7_cm.__exit__(None, None, None)
        with tc.tile_pool(name="ps_r4", bufs=1, space="PSUM") as PSR4, \
             tc.tile_pool(name="tmp4", bufs=2) as TMP4:
            _ln_full(nc, tc, TMP4, PSR4, rows, x1f, x1f, None, ones32,
                     blob, "n2w", "n2b", None, None, cst[0:1, 0:1])
        TMP3_cm.__exit__(None, None, None)
        nc.sync.dma_start(out=t_out.ap(), in_=x1f[:])


def _spline_rows(nc, rs, sh, sh2, epsap):
    """rs(7) = mu_h, rs(8) = S = 1/(sqrt(var+eps)*(norm+1)),
    norm = sqrt(FF*var/(var+eps) + eps)."""
    v, s = nc.vector, nc.scalar
    mu = rs(7)
    S_ = rs(8)
    var = rs(13)
    t1 = rs(14)
    t2 = rs(15)
    # S = 1/(sqrt(a)*(1+norm)) = 1/(sqrt(a) + sqrt(FF*var + eps*a)),
    # a = var+eps, norm = sqrt(FF*var/a + eps) (approx, eps*norm^2 ~ 0)
    v.tensor_scalar(mu, sh, 1.0 / FF, None, Alu.mult)
    v.tensor_tensor(var, mu, mu, Alu.mult)
    v.tensor_scalar(t1, sh2, 1.0 / FF, None, Alu.mult)
    v.tensor_tensor(var, t1, var, Alu.subtract)
    v.tensor_scalar(t1, var, EPS, None, Alu.add)      # a
    s.activation(t2, t1, Act.Sqrt)                    # sqrt(a)
    v.tensor_scalar(t1, t1, EPS, None, Alu.mult)
    v.tensor_scalar(var, var, float(FF), None, Alu.mult)
    v.tensor_tensor(t1, t1, var, Alu.add)             # FF*var + eps*a
    s.activation(t1, t1, Act.Sqrt)
    v.tensor_tensor(t2, t2, t1, Alu.add)
    v.reciprocal(S_, t2)


def _ln_full(nc, tc, TMP, PSR, rows, src, dstf, dst8, ones32, blob,
             wnm, bnm, w16nm, b16nm, epsap):
    co = lambda nm, k: blob[:, _CO[nm] + k:_CO[nm] + k + 1]
    v, s, g, te = nc.vector, nc.scalar, nc.gpsimd, nc.tensor
    T = TOK
    rs = lambda k: rows[0:1, k * T:(k + 1) * T]
    t_sx = PSR.tile([1, T], F32, tag="lnsxp")
    t_sx2 = PSR.tile([1, T], F32, tag="lnsx2p")
    sx, sx2 = t_sx[:], t_sx2[:]
    onesbl = TMP.tile([128, 1], mybir.dt.bfloat16, tag="lnonesb")
    v.memset(onesbl[:], 1.0)
    srcb = TMP.tile([128, 8 * T], mybir.dt.bfloat16, tag="lnsrcb")
    for kc in range(8):
        (v if kc % 2 == 0 else g).tensor_copy(
            srcb[:, T * kc:T * (kc + 1)], src[:, T * kc:T * (kc + 1)])
    for kc in range(8):
        te.matmul(sx, onesbl[:], srcb[:, T * kc:T * (kc + 1)],
                  start=(kc == 0), stop=(kc == 7))
    xsq = TMP.tile([128, T], mybir.dt.bfloat16, tag="lnxsq")
    for kc in range(8):
        v.tensor_tensor(xsq[:], srcb[:, T * kc:T * (kc + 1)],
                        srcb[:, T * kc:T * (kc + 1)], Alu.mult)
        te.matmul(sx2, onesbl[:], xsq[:], start=(kc == 0), stop=(kc == 7))
    _ln_rows(nc, rs(9), rs(10), rs(11), sx, sx2, D, epsap)
    mu_b = TMP.tile([128, T], F32, tag="lnmu_b")
    s_b = TMP.tile([128, T], F32, tag="lns_b")
    g.partition_broadcast(mu_b[:], rs(9))
    g.partition_broadcast(s_b[:], rs(10))
    tm = TMP.tile([128, T], F32, tag="lntm")
    for kc in range(8):
        v.tensor_tensor(tm[:], src[:, T * kc:T * (kc + 1)], mu_b[:],
                        Alu.subtract)
        v.tensor_tensor(tm[:], tm[:], s_b[:], Alu.mult)
        v.tensor_scalar(dstf[:, T * kc:T * (kc + 1)], tm[:],
                        co(wnm, kc), co(bnm, kc),
                        Alu.mult, Alu.add)
        if dst8 is not None:
            s.activation(dst8[:, kc:kc + 1, :], tm[:], Act.Identity,
                         bias=co(b16nm, kc), scale=co(w16nm, kc))


# ----------------------------------------------------------------------------
# Entry point
# ----------------------------------------------------------------------------

def kernel(**inputs):
    in_maps, sc = _prepare_inputs(inputs)
    key = hashlib.sha256(
        repr((sc["coeffs"], sc["ent_b"], sc["ep2_b"])).encode()
    ).hexdigest()
    if key not in _prog_cache:
        _prog_cache[key] = _build_program(sc)
    nc = _prog_cache[key]
    res = bass_utils.run_bass_kernel_spmd(nc, in_maps,
                                          core_ids=list(range(NCORES)))
    out = np.empty((1, S, D), np.float32)
    for c in range(NCORES):
        oc = np.asarray(res.results[c]["out"], np.float32)   # [128, 8*TOK]
        ot = oc.reshape(128, 8, TOK).transpose(1, 0, 2).reshape(D, TOK)
        out[0, c * TOK:(c + 1) * TOK, :] = ot.T
    return out
